# revision 10
# baseline (speedup 1.0000x reference)
"""Dilated segment attention on 8 TRN2 NeuronCores (Bass/Tile).

Problem (hardcoded from spec):
  x [2, 8192, 2048] f32, Wqkv [6144, 2048], b_qkv [6144], Wout [2048, 2048],
  b_out [2048].  segment=512, dilation=2 -> 16 segments of L=256 dilated
  tokens per batch; per-segment 16-head attention (hd=128); fused qkv and
  out projections.  Output [2, 4096, 2048] f32.

Sharding: the 32 (batch, segment) instances are independent -> 4 per core.
Host pre-gathers the dilated tokens, pre-transposes/pre-tiles operands and
casts to bf16 (compute precision; measured end-to-end rel err ~5e-3).

Per-core dataflow (all matmuls K=128, bf16):
  q/k proj   : feature-major  qkT[e, tok] = W-tile.T @ xsT-tile (accum 16 d-tiles)
  v proj     : token-major    v[tok, e]   = xsT-tile.T @ WvT-quarter
               (drains land directly in the AV-stationary layout — no
               transposes; DMA transposes act as global DMA barriers in
               Tile's deadlock-avoidance and measurably stall the PE)
  scores     : scoresT[lk, lq] = kT.T @ qT  (per seg, head; operands swapped)
  softmax    : exp on ScalarE (scale=1/sqrt(hd); scores provably in [-6, 6]
               so no max subtraction), sums via ones-matmul, normalize on DVE
  AV         : outT[hd, lq] = v[lk, hd].T @ expT[lk, lq]
  out proj   : out[l, e] = aT-tile.T @ WoutT-tile  (accum 16 head-tiles,
               token-major, so the HBM store is linear)
b_qkv's q/k parts are applied on-chip (they feed the softmax); its v part
and b_out are applied on the host: softmax weights sum to 1, so a v bias
shifts the attention output by exactly b_v, i.e. out += Wout @ b_v + b_out.

Schedule (v3 — trace-driven):
  - ~12 zero matmuls at kernel start (overlapping the input DMA wait) keep
    the PE HAM busy-window warm so real matmuls run at 2.4GHz immediately.
  - All startup loads on one HWDGE ring, interleaved [wv0/4, xq0, wv0/4,
    xq1, ...] so the first v-projection group ramps with the arrivals.
  - psum->sbuf drains are split/alternated ScalarE / DVE so chunk-boundary
    psum WAR frees faster and neither engine's FIFO gates the PE.
  - attention for head h-1 is interleaved between head h's projection
    matmuls (1 step per 3 d-tiles): the exp ACT latency and the per-tile
    LDWEIGHTS hide under 216ns projection matmuls.  Head 15's attention
    interleaves into the first out-projection psum groups.
"""

import numpy as np
import ml_dtypes

B = 2
S = 8192
D = 2048
H = 16
HD = 128
SEGMENT = 512
DIL = 2
NSEG = S // SEGMENT          # 16
L = SEGMENT // DIL           # 256 dilated tokens per segment
N_CORES = 8
PAIRS = B * NSEG             # 32 independent (b, n) instances
SPC = PAIRS // N_CORES       # 4 segments per core
TOK = SPC * L                # 1024 tokens per core
DT = D // 128                # 16 contraction tiles
NQK = 2 * D // 128           # 32 q/k feature chunks (16 q, 16 k)
SCALE = 1.0 / float(np.sqrt(HD))

_PROGRAM = None


def _build_program():
    import concourse.bass as bass
    import concourse.bacc as bacc
    import concourse.tile as tile
    from concourse import mybir

    BF = mybir.dt.bfloat16
    F32 = mybir.dt.float32
    ts = bass.ts
    IDENT = mybir.ActivationFunctionType.Identity
    EXP = mybir.ActivationFunctionType.Exp

    nc = bacc.Bacc("TRN2", target_bir_lowering=False, debug=False,
                   num_devices=N_CORES)

    xst_d = nc.dram_tensor("xst", [128, DT * TOK], BF, kind="ExternalInput")
    wqkv_d = nc.dram_tensor("wqkv_t", [NQK, 128, DT * 128], BF,
                            kind="ExternalInput")
    wv_d = nc.dram_tensor("wv_t", [4, 128, DT * 512], BF, kind="ExternalInput")
    wout_d = nc.dram_tensor("wout_t", [4, 128, DT * 512], BF, kind="ExternalInput")
    bq_d = nc.dram_tensor("bq_t", [128, NQK], F32, kind="ExternalInput")
    out_d = nc.dram_tensor("out", [TOK, D], F32, kind="ExternalOutput")

    with tile.TileContext(nc) as tc:
        with (
            tc.tile_pool(name="const", bufs=1) as const_p,
            tc.tile_pool(name="big", bufs=1) as big_p,
            tc.tile_pool(name="wq", bufs=6) as w_p,
            tc.tile_pool(name="qk", bufs=4) as qk_p,
            tc.tile_pool(name="ex", bufs=4) as ex_p,
            tc.tile_pool(name="st", bufs=2) as st_p,
            tc.tile_pool(name="ou", bufs=3) as ou_p,
            tc.tile_pool(name="pp", bufs=4, space="PSUM") as pp_p,
            tc.tile_pool(name="pa", bufs=2, space="PSUM") as pa_p,
        ):
            ones = const_p.tile([128, 1], BF)
            nc.gpsimd.memset(ones[:], 1.0)
            # PE warmup: the HAM clock gate holds the PE at 1.2GHz until
            # ~3.4us of sustained activity.  Burn that window on zero
            # matmuls while the input DMAs stream, so the first real
            # matmul runs at 2.4GHz.
            warm_sb = const_p.tile([128, 512], BF)
            nc.gpsimd.memset(warm_sb[:], 0.0)
            warm_ps = pp_p.tile([128, 512], F32, tag="pp", name="warm")
            for i in range(16):
                nc.tensor.matmul(warm_ps[:], warm_sb[:, 0:128], warm_sb[:],
                                 start=(i == 0), stop=(i == 15))

            # Startup loads: one FIFO ring, v-quarter-0 slices interleaved
            # with the xst quarters so the first v group's d-tile matmuls
            # start as each (weights, activations) pair lands.
            wv_tiles = [None] * 4
            wv_tiles[0] = w_p.tile([128, DT, 512], BF, tag="wo", bufs=2,
                                   name="wv_t")
            xq_sb = [big_p.tile([128, 4, TOK], BF, name=f"xq{k}")
                     for k in range(4)]
            bq_sb = const_p.tile([128, NQK], F32)
            nc.scalar.dma_start(out=bq_sb[:], in_=bq_d[:])
            for k in range(4):
                nc.sync.dma_start(out=wv_tiles[0][:, 4 * k:4 * (k + 1), :],
                                  in_=wv_d[0][:, 4 * k * 512:4 * (k + 1) * 512])
                nc.sync.dma_start(out=xq_sb[k][:],
                                  in_=xst_d[:, 4 * k * TOK:4 * (k + 1) * TOK])
            vtok_sb = big_p.tile([128, H, SPC * 2, 128], BF)
            aT_sb = big_p.tile([128, SPC, H, L], BF)

            # ---- v projection, token-major (out-proj-style groups) ----
            # vtok[p, h, t, j] = v[tok = t*128+p, hd = j]: each (vc, t)
            # group accumulates [128 tok, 512 v-feats] over the 16 d-tiles
            # and drains straight into the AV-stationary layout.
            for vc in range(4):
                if vc + 1 < 4:
                    wv_tiles[vc + 1] = w_p.tile([128, DT, 512], BF, tag="wo",
                                                bufs=2, name="wv_t")
                    nc.sync.dma_start(out=wv_tiles[vc + 1][:],
                                      in_=wv_d[vc + 1])
                for t in range(SPC * 2):
                    vps = pp_p.tile([128, 512], F32, tag="pp", name="vps")
                    for dt in range(DT):
                        q, r = divmod(dt, 4)
                        nc.tensor.matmul(
                            vps[:],
                            xq_sb[q][:, r, t * 128:(t + 1) * 128],
                            wv_tiles[vc][:, dt, :],
                            start=(dt == 0),
                            stop=(dt == DT - 1),
                        )
                    dst = vtok_sb[:, 4 * vc:4 * (vc + 1), t, :]
                    if t % 2:
                        nc.vector.tensor_copy(out=dst, in_=vps[:])
                    else:
                        nc.scalar.activation(out=dst, in_=vps[:], func=IDENT,
                                             scale=1.0)

            def proj_steps(c, out_tile):
                """q/k chunk c: out_tile[128, TOK] bf16 = (W chunk).T @ xsT + b.

                Generator: yields after each d-tile's matmul pair; emits the
                split-engine psum drains on exhaustion.
                """
                wck = w_p.tile([128, DT * 128], BF, tag="w")
                nc.sync.dma_start(out=wck[:], in_=wqkv_d[c])
                pss = [pp_p.tile([128, 512], F32, tag="pp", name=f"ps{half}")
                       for half in range(2)]
                for dt in range(DT):
                    q, r = divmod(dt, 4)
                    for half in range(2):
                        nc.tensor.matmul(
                            pss[half][:],
                            wck[:, ts(dt, 128)],
                            xq_sb[q][:, r, ts(half, 512)],
                            start=(dt == 0),
                            stop=(dt == DT - 1),
                        )
                    yield
                nc.scalar.activation(
                    out=out_tile[:, ts(0, 512)], in_=pss[0][:],
                    func=IDENT, bias=bq_sb[:, c:c + 1], scale=1.0,
                )
                nc.vector.tensor_scalar_add(
                    out_tile[:, ts(1, 512)], pss[1][:], bq_sb[:, c:c + 1],
                )

            # ---- per-head attention, interleaved into the next head's
            # projection matmul stream ----
            def attention_thunks(h, qh, kh):
                """16 emission steps for head h's attention over 4 segments.

                Each step issues at most ONE new PE stationary (scores pair
                shares the q moving operand; sums pair shares the ones
                stationary after LDWEIGHTS dedupe), so with >=2 projection
                matmuls between steps every attention LDWEIGHTS loads into
                the background weight buffer under a 216ns projection
                matmul instead of stalling the PE (measured +90..160ns on
                every av-group matmul when sums+AV+AV issued back-to-back).
                """
                e_ts = [None] * SPC
                avss = [None] * SPC

                def sc_step(seg):
                    def emit():
                        scT = pa_p.tile([128, 2, L], F32, tag="pa", name="scT")
                        for lkc in range(2):
                            nc.tensor.matmul(
                                scT[:, lkc, :],
                                kh[:, seg * L + lkc * 128:
                                   seg * L + (lkc + 1) * 128],
                                qh[:, seg * L:(seg + 1) * L],
                            )
                        e_t = ex_p.tile([128, 2, L], BF, tag="ex")
                        nc.scalar.activation(out=e_t[:], in_=scT[:],
                                             func=EXP, scale=SCALE)
                        e_ts[seg] = e_t
                    return emit

                def sums_step(seg):
                    def emit():
                        # av ([:, 0, :]) and the softmax sums row
                        # ([0:1, 1, :]) share one PSUM bank; Tile
                        # serializes the cross-use.
                        avs = avss[seg] = pa_p.tile([128, 2, L], F32,
                                                    tag="pav", bufs=2,
                                                    name="avs")
                        for lkc in range(2):
                            nc.tensor.matmul(
                                avs[0:1, 1, :], ones[:], e_ts[seg][:, lkc, :],
                                start=(lkc == 0), stop=(lkc == 1),
                            )
                    return emit

                def av_step(seg, lkc):
                    def emit():
                        nc.tensor.matmul(
                            avss[seg][:, 0, :],
                            vtok_sb[:, h, seg * 2 + lkc, :],
                            e_ts[seg][:, lkc, :],
                            start=(lkc == 0), stop=(lkc == 1),
                        )
                        if lkc == 1:
                            avs = avss[seg]
                            inv = st_p.tile([1, L], F32, tag="st")
                            nc.vector.reciprocal_approx_fast(
                                out=inv[:], in_=avs[0:1, 1, :])
                            invB = ex_p.tile([128, L], F32, tag="invb")
                            nc.gpsimd.partition_broadcast(invB[:], inv[:])
                            nc.vector.tensor_mul(aT_sb[:, seg, h, :],
                                                 avs[:, 0, :], invB[:])
                    return emit

                return [sc_step(0), sc_step(1),
                        sums_step(0), av_step(0, 0), av_step(0, 1),
                        sc_step(2),
                        sums_step(1), av_step(1, 0), av_step(1, 1),
                        sc_step(3),
                        sums_step(2), av_step(2, 0), av_step(2, 1),
                        sums_step(3), av_step(3, 0), av_step(3, 1)]

            def run_interleaved(gens, thunks, every):
                k, ai = 0, 0
                for g in gens:
                    for _ in g:
                        k += 1
                        if k % every == 0 and ai < len(thunks):
                            thunks[ai]()
                            ai += 1
                while ai < len(thunks):
                    thunks[ai]()
                    ai += 1

            wq_eq0 = None
            prev_qk = None
            for h in range(H):
                if h == H - 1:
                    # Prefetch the first Wout quarter one head early so the
                    # out-projection never waits on its 2MB load.
                    wq_eq0 = w_p.tile([128, DT, 512], BF, tag="wo", bufs=2,
                                      name="wq_t")
                    nc.sync.dma_start(out=wq_eq0[:], in_=wout_d[0])
                qh = qk_p.tile([128, TOK], BF, tag="qk")
                kh = qk_p.tile([128, TOK], BF, tag="qk")
                gens = [proj_steps(h, qh), proj_steps(16 + h, kh)]
                thunks = (attention_thunks(h - 1, *prev_qk)
                          if prev_qk is not None else [])
                run_interleaved(gens, thunks, every=2)
                prev_qk = (qh, kh)
            last_attn = attention_thunks(H - 1, *prev_qk)

            # ---- output projection (token-major) ----
            # Wout streamed in four 2MB e-quarters; head 15's attention
            # steps interleave into the first psum group (its aT d-tile is
            # the last accumulated, so each segment's normalize only has to
            # beat d-tile 15 of its own token tile).
            def po_steps(eq, lc, wq_t):
                seg, lqc = lc // 2, lc % 2
                po = pp_p.tile([128, 512], F32, tag="pp", name="po")
                for dt in range(DT):
                    nc.tensor.matmul(
                        po[:],
                        aT_sb[:, seg, dt, ts(lqc, 128)],
                        wq_t[:, dt, :],
                        start=(dt == 0),
                        stop=(dt == DT - 1),
                    )
                    yield
                ob = ou_p.tile([128, 512], F32, tag="ou")
                if lc % 2:
                    nc.vector.tensor_copy(out=ob[:], in_=po[:])
                else:
                    nc.scalar.activation(out=ob[:], in_=po[:], func=IDENT,
                                         scale=1.0)
                nc.sync.dma_start(
                    out=out_d[lc * 128:(lc + 1) * 128,
                              eq * 512:(eq + 1) * 512],
                    in_=ob[:],
                )

            for eq in range(4):
                if eq == 0:
                    wq_t = wq_eq0
                else:
                    wq_t = w_p.tile([128, DT, 512], BF, tag="wo", bufs=2,
                                    name="wq_t")
                    nc.sync.dma_start(out=wq_t[:], in_=wout_d[eq])
                for lc in range(TOK // 128):
                    if eq == 0 and lc < 2:
                        thunks = last_attn[8 * lc:8 * (lc + 1)]
                    else:
                        thunks = []
                    run_interleaved([po_steps(eq, lc, wq_t)], thunks,
                                    every=2)

    nc.compile()
    _dedupe_ldweights(nc)
    return nc


def _dedupe_ldweights(nc):
    """Drop InstLdweights whose weights are already resident in the PE array.

    tile_legalize emits one LDWEIGHTS per matmul; consecutive matmuls that
    share the stationary operand (projection token-halves) reload identical
    weights, costing ~97ns of PE pipe each.  Walk each block's PE stream
    tracking the loaded-weights key and delete reloads.  Only semaphore-free
    LDWEIGHTS are dropped, so the sync graph is untouched;
    EVENT_SEMAPHORE/DRAIN between pairs don't disturb the array, any other
    PE instruction conservatively invalidates the key.
    """
    from concourse import mybir

    PE = mybir.EngineType.PE
    dropped = 0
    for f in nc.m.functions:
        for blk in f.blocks:
            insts = blk.instructions
            loaded = None
            to_drop = []
            for idx, x in enumerate(insts):
                if getattr(x, "engine", None) != PE:
                    continue
                nm = type(x).__name__
                if nm == "InstLdweights":
                    si = x.sync_info
                    clean = si is None or (not si.on_wait and not si.on_update)
                    key = (str(x.ins[0]), str(x.is_transpose),
                           str(x.perf_mode), str(x.tile_position))
                    if clean and loaded == key:
                        to_drop.append(idx)
                    else:
                        loaded = key
                elif nm == "InstMatmult":
                    continue
                elif nm in ("InstEventSemaphore", "InstDrain"):
                    continue
                else:
                    loaded = None
            for idx in reversed(to_drop):
                del insts[idx]
            blk.instructions = insts
            dropped += len(to_drop)
    return dropped


def get_program():
    global _PROGRAM
    if _PROGRAM is None:
        _PROGRAM = _build_program()
    return _PROGRAM


def make_in_maps(x, Wqkv, b_qkv):
    """Host-side shard + layout prep (bf16 casts, transposes, tiling)."""
    bf16 = ml_dtypes.bfloat16
    x = np.asarray(x, dtype=np.float32)
    Wqkv = np.asarray(Wqkv, dtype=np.float32)
    b_qkv = np.asarray(b_qkv, dtype=np.float32)

    xs = x.reshape(B, NSEG, SEGMENT, D)[:, :, ::DIL, :]     # [2,16,256,2048]
    xs_flat = xs.reshape(PAIRS, L, D)

    # q/k lhsT tiles packed partition-major: wt[c, p, dt*128+j] =
    # WqkvT[dt*128+p, c*128+j] so one chunk is a single linear DMA.
    wt = np.ascontiguousarray(
        Wqkv[:2 * D].reshape(NQK, 128, DT, 128).transpose(0, 3, 2, 1)
        .reshape(NQK, 128, DT * 128)
    ).astype(bf16)                                          # [32,128,2048]
    # v moving quarters, same layout as the Wout quarters.
    wvt = _quarter_tiles(Wqkv[2 * D:])                      # [4,128,DT*512]
    bqt = np.ascontiguousarray(b_qkv[:2 * D].reshape(NQK, 128).T)  # [128,32]

    in_maps = []
    for i in range(N_CORES):
        tok = xs_flat[SPC * i:SPC * (i + 1)].reshape(TOK, D)
        xst = np.ascontiguousarray(
            tok.T.reshape(DT, 128, TOK).transpose(1, 0, 2)
            .reshape(128, DT * TOK)).astype(bf16)
        in_maps.append({"xst": xst, "wqkv_t": wt, "wv_t": wvt, "bq_t": bqt})
    return in_maps


def _quarter_tiles(W):
    """[2048, 2048] row-major -> [eq, p, dt*512+j] = W[eq*512+j, dt*128+p]."""
    W = np.asarray(W, dtype=np.float32)
    return np.ascontiguousarray(
        W.T.reshape(DT, 128, 4, 512).transpose(2, 1, 0, 3)
        .reshape(4, 128, DT * 512)).astype(ml_dtypes.bfloat16)


def make_wout_tiled(Wout):
    return _quarter_tiles(Wout)


def kernel(x, Wqkv, b_qkv, Wout, b_out):
    from concourse import bass_utils

    nc = get_program()
    in_maps = make_in_maps(x, Wqkv, b_qkv)
    wot = make_wout_tiled(Wout)
    for m in in_maps:
        m["wout_t"] = wot

    res = bass_utils.run_bass_kernel_spmd(
        nc, in_maps, core_ids=list(range(N_CORES)))
    outs = [res.results[i]["out"] for i in range(N_CORES)]
    # Softmax weights sum to 1, so the v bias passes through attention
    # unchanged: out += Wout @ b_v + b_out (host-side, exact).
    bias = (np.asarray(Wout, dtype=np.float32)
            @ np.asarray(b_qkv, dtype=np.float32)[2 * D:]
            + np.asarray(b_out, dtype=np.float32))
    full = np.concatenate(outs, axis=0) + bias
    return np.ascontiguousarray(full.reshape(B, NSEG * L, D), dtype=np.float32)


# revision 17
# speedup vs baseline: 1.0025x; 1.0025x over previous
"""Dilated segment attention on 8 TRN2 NeuronCores (Bass/Tile).

Problem (hardcoded from spec):
  x [2, 8192, 2048] f32, Wqkv [6144, 2048], b_qkv [6144], Wout [2048, 2048],
  b_out [2048].  segment=512, dilation=2 -> 16 segments of L=256 dilated
  tokens per batch; per-segment 16-head attention (hd=128); fused qkv and
  out projections.  Output [2, 4096, 2048] f32.

Sharding: the 32 (batch, segment) instances are independent -> 4 per core.
Host pre-gathers the dilated tokens, pre-transposes/pre-tiles operands and
casts to bf16 (compute precision; measured end-to-end rel err ~5e-3).

Per-core dataflow (all matmuls K=128, bf16):
  q/k proj   : feature-major  qkT[e, tok] = W-tile.T @ xsT-tile (accum 16 d-tiles)
  v proj     : token-major    v[tok, e]   = xsT-tile.T @ WvT-quarter
               (drains land directly in the AV-stationary layout — no
               transposes; DMA transposes act as global DMA barriers in
               Tile's deadlock-avoidance and measurably stall the PE)
  scores     : scoresT[lk, lq] = kT.T @ qT  (per seg, head; operands swapped)
  softmax    : exp on ScalarE (scale=1/sqrt(hd); scores provably in [-6, 6]
               so no max subtraction), sums via ones-matmul, normalize on DVE
  AV         : outT[hd, lq] = v[lk, hd].T @ expT[lk, lq]
  out proj   : out[l, e] = aT-tile.T @ WoutT-tile  (accum 16 head-tiles,
               token-major, so the HBM store is linear)
b_qkv's q/k parts are applied on-chip (they feed the softmax); its v part
and b_out are applied on the host: softmax weights sum to 1, so a v bias
shifts the attention output by exactly b_v, i.e. out += Wout @ b_v + b_out.

Schedule (v3 — trace-driven):
  - ~12 zero matmuls at kernel start (overlapping the input DMA wait) keep
    the PE HAM busy-window warm so real matmuls run at 2.4GHz immediately.
  - All startup loads on one HWDGE ring, interleaved [wv0/4, xq0, wv0/4,
    xq1, ...] so the first v-projection group ramps with the arrivals.
  - psum->sbuf drains are split/alternated ScalarE / DVE so chunk-boundary
    psum WAR frees faster and neither engine's FIFO gates the PE.
  - attention for head h-1 is interleaved between head h's projection
    matmuls (1 step per 3 d-tiles): the exp ACT latency and the per-tile
    LDWEIGHTS hide under 216ns projection matmuls.  Head 15's attention
    interleaves into the first out-projection psum groups.
"""

import numpy as np
import ml_dtypes

B = 2
S = 8192
D = 2048
H = 16
HD = 128
SEGMENT = 512
DIL = 2
NSEG = S // SEGMENT          # 16
L = SEGMENT // DIL           # 256 dilated tokens per segment
N_CORES = 8
PAIRS = B * NSEG             # 32 independent (b, n) instances
SPC = PAIRS // N_CORES       # 4 segments per core
TOK = SPC * L                # 1024 tokens per core
DT = D // 128                # 16 contraction tiles
NQK = 2 * D // 128           # 32 q/k feature chunks (16 q, 16 k)
SCALE = 1.0 / float(np.sqrt(HD))

_PROGRAM = None


def _build_program():
    import concourse.bass as bass
    import concourse.bacc as bacc
    import concourse.tile as tile
    from concourse import mybir

    BF = mybir.dt.bfloat16
    F32 = mybir.dt.float32
    ts = bass.ts
    IDENT = mybir.ActivationFunctionType.Identity
    EXP = mybir.ActivationFunctionType.Exp

    nc = bacc.Bacc("TRN2", target_bir_lowering=False, debug=False,
                   num_devices=N_CORES)

    xst_d = nc.dram_tensor("xst", [128, DT * TOK], BF, kind="ExternalInput")
    wqkv_d = nc.dram_tensor("wqkv_t", [NQK, 128, DT * 128], BF,
                            kind="ExternalInput")
    wv_d = nc.dram_tensor("wv_t", [4, 128, DT * 512], BF, kind="ExternalInput")
    wout_d = nc.dram_tensor("wout_t", [4, 128, DT * 512], BF, kind="ExternalInput")
    bq_d = nc.dram_tensor("bq_t", [128, NQK], F32, kind="ExternalInput")
    out_d = nc.dram_tensor("out", [TOK, D], F32, kind="ExternalOutput")

    with tile.TileContext(nc) as tc:
        with (
            tc.tile_pool(name="const", bufs=1) as const_p,
            tc.tile_pool(name="big", bufs=1) as big_p,
            tc.tile_pool(name="wq", bufs=6) as w_p,
            tc.tile_pool(name="qk", bufs=4) as qk_p,
            tc.tile_pool(name="ex", bufs=4) as ex_p,
            tc.tile_pool(name="st", bufs=2) as st_p,
            tc.tile_pool(name="ou", bufs=3) as ou_p,
            tc.tile_pool(name="pp", bufs=4, space="PSUM") as pp_p,
            tc.tile_pool(name="pa", bufs=2, space="PSUM") as pa_p,
        ):
            ones = const_p.tile([128, 1], BF)
            nc.gpsimd.memset(ones[:], 1.0)
            # PE warmup: the HAM clock gate holds the PE at 1.2GHz until
            # ~3.4us of sustained activity.  Burn that window on zero
            # matmuls while the input DMAs stream, so the first real
            # matmul runs at 2.4GHz.
            warm_sb = const_p.tile([128, 512], BF)
            nc.gpsimd.memset(warm_sb[:], 0.0)
            warm_ps = pp_p.tile([128, 512], F32, tag="pp", name="warm")
            for i in range(16):
                nc.tensor.matmul(warm_ps[:], warm_sb[:, 0:128], warm_sb[:],
                                 start=(i == 0), stop=(i == 15))

            # Startup loads: one FIFO ring, v-quarter-0 slices interleaved
            # with the xst quarters so the first v group's d-tile matmuls
            # start as each (weights, activations) pair lands.
            wv_tiles = [None] * 4
            wv_tiles[0] = w_p.tile([128, DT, 512], BF, tag="wo", bufs=2,
                                   name="wv_t")
            xq_sb = [big_p.tile([128, 4, TOK], BF, name=f"xq{k}")
                     for k in range(4)]
            bq_sb = const_p.tile([128, NQK], F32)
            nc.scalar.dma_start(out=bq_sb[:], in_=bq_d[:])
            for k in range(4):
                nc.sync.dma_start(out=wv_tiles[0][:, 4 * k:4 * (k + 1), :],
                                  in_=wv_d[0][:, 4 * k * 512:4 * (k + 1) * 512])
                nc.sync.dma_start(out=xq_sb[k][:],
                                  in_=xst_d[:, 4 * k * TOK:4 * (k + 1) * TOK])
            vtok_sb = big_p.tile([128, H, SPC * 2, 128], BF)
            aT_sb = big_p.tile([128, SPC, H, L], BF)

            # ---- v projection, token-major (out-proj-style groups) ----
            # vtok[p, h, t, j] = v[tok = t*128+p, hd = j]: each (vc, t)
            # group accumulates [128 tok, 512 v-feats] over the 16 d-tiles
            # and drains straight into the AV-stationary layout.
            def v_drain(vc, t, vps):
                dst = vtok_sb[:, 4 * vc:4 * (vc + 1), t, :]
                if t % 2:
                    nc.vector.tensor_copy(out=dst, in_=vps[:])
                else:
                    nc.scalar.activation(out=dst, in_=vps[:], func=IDENT,
                                         scale=1.0)

            # Double-buffered wv prefetch: vc+1's 2MB load is dispatched at
            # the start of vc's compute.
            wv_tiles[1] = w_p.tile([128, DT, 512], BF, tag="wo", bufs=2,
                                   name="wv_t")
            nc.sync.dma_start(out=wv_tiles[1][:], in_=wv_d[1])

            # vc0 rides the input-DMA ramp: sweep d-tile-quarter-major
            # across 4 concurrent token-tile psum groups, so every quarter
            # arrival unlocks 16 matmuls instead of 4 (the in-order PE
            # stream would otherwise head-of-line block on group 0's next
            # quarter while groups 1-7's early d-tiles are ready).
            for wave in range(2):
                vpss = [pp_p.tile([128, 512], F32, tag="pp", name="vps")
                        for _ in range(4)]
                for qk in range(4):
                    for ti in range(4):
                        for r in range(4):
                            nc.tensor.matmul(
                                vpss[ti][:],
                                xq_sb[qk][:, r, (4 * wave + ti) * 128:
                                          (4 * wave + ti + 1) * 128],
                                wv_tiles[0][:, 4 * qk + r, :],
                                start=(qk == 0 and r == 0),
                                stop=(qk == 3 and r == 3),
                            )
                for ti in range(4):
                    v_drain(0, 4 * wave + ti, vpss[ti])

            for vc in range(1, 4):
                if vc + 1 < 4:
                    wv_tiles[vc + 1] = w_p.tile([128, DT, 512], BF, tag="wo",
                                                bufs=2, name="wv_t")
                    nc.sync.dma_start(out=wv_tiles[vc + 1][:],
                                      in_=wv_d[vc + 1])
                for t in range(SPC * 2):
                    vps = pp_p.tile([128, 512], F32, tag="pp", name="vps")
                    for dt in range(DT):
                        q, r = divmod(dt, 4)
                        nc.tensor.matmul(
                            vps[:],
                            xq_sb[q][:, r, t * 128:(t + 1) * 128],
                            wv_tiles[vc][:, dt, :],
                            start=(dt == 0),
                            stop=(dt == DT - 1),
                        )
                    v_drain(vc, t, vps)

            def proj_steps(c, out_tile):
                """q/k chunk c: out_tile[128, TOK] bf16 = (W chunk).T @ xsT + b.

                Generator: yields after each d-tile's matmul pair; emits the
                split-engine psum drains on exhaustion.
                """
                wck = w_p.tile([128, DT * 128], BF, tag="w")
                nc.sync.dma_start(out=wck[:], in_=wqkv_d[c])
                pss = [pp_p.tile([128, 512], F32, tag="pp", name=f"ps{half}")
                       for half in range(2)]
                for dt in range(DT):
                    q, r = divmod(dt, 4)
                    for half in range(2):
                        nc.tensor.matmul(
                            pss[half][:],
                            wck[:, ts(dt, 128)],
                            xq_sb[q][:, r, ts(half, 512)],
                            start=(dt == 0),
                            stop=(dt == DT - 1),
                        )
                    yield
                nc.scalar.activation(
                    out=out_tile[:, ts(0, 512)], in_=pss[0][:],
                    func=IDENT, bias=bq_sb[:, c:c + 1], scale=1.0,
                )
                nc.vector.tensor_scalar_add(
                    out_tile[:, ts(1, 512)], pss[1][:], bq_sb[:, c:c + 1],
                )

            # ---- per-head attention, interleaved into the next head's
            # projection matmul stream ----
            def attention_thunks(h, qh, kh):
                """16 emission steps for head h's attention over 4 segments.

                Each step issues at most ONE new PE stationary (scores pair
                shares the q moving operand; sums pair shares the ones
                stationary after LDWEIGHTS dedupe), so with >=2 projection
                matmuls between steps every attention LDWEIGHTS loads into
                the background weight buffer under a 216ns projection
                matmul instead of stalling the PE (measured +90..160ns on
                every av-group matmul when sums+AV+AV issued back-to-back).
                """
                e_ts = [None] * SPC
                scts = [None] * SPC
                avss = [None] * SPC
                invBs = [None] * SPC

                def sc_step(seg):
                    def emit():
                        scT = scts[seg] = pa_p.tile([128, 2, L], F32,
                                                    tag="pa", name="scT")
                        for lkc in range(2):
                            nc.tensor.matmul(
                                scT[:, lkc, :],
                                kh[:, seg * L + lkc * 128:
                                   seg * L + (lkc + 1) * 128],
                                qh[:, seg * L:(seg + 1) * L],
                            )
                        e_t = ex_p.tile([128, 2, L], BF, tag="ex")
                        nc.scalar.activation(out=e_t[:], in_=scT[:],
                                             func=EXP, scale=SCALE)
                        e_ts[seg] = e_t
                    return emit

                def sums_step(seg):
                    def emit():
                        # One N=512 matmul; the per-chunk sums land in the
                        # already-consumed scores psum rows (WAR on the exp
                        # read, which the e_t input implies anyway).  DVE
                        # adds the two chunk sums and inverts.
                        scT = scts[seg]
                        nc.tensor.matmul(
                            scT[0:1, :, :], ones[:], e_ts[seg][:, :, :],
                        )
                        s_sb = st_p.tile([1, 2, L], F32, tag="s2")
                        nc.scalar.activation(out=s_sb[:], in_=scT[0:1, :, :],
                                             func=IDENT, scale=1.0)
                        s_sum = st_p.tile([1, L], F32, tag="st")
                        nc.vector.tensor_add(s_sum[:], s_sb[0:1, 0, :],
                                             s_sb[0:1, 1, :])
                        s_inv = st_p.tile([1, L], F32, tag="st")
                        nc.vector.reciprocal_approx_fast(out=s_inv[:],
                                                         in_=s_sum[:])
                        invB = invBs[seg] = ex_p.tile([128, L], F32,
                                                      tag="invb", name="invB")
                        nc.gpsimd.partition_broadcast(invB[:], s_inv[:])
                        avss[seg] = pa_p.tile([128, L], F32, tag="pav",
                                              bufs=2, name="avs")
                    return emit

                def av_step(seg, lkc):
                    def emit():
                        nc.tensor.matmul(
                            avss[seg][:],
                            vtok_sb[:, h, seg * 2 + lkc, :],
                            e_ts[seg][:, lkc, :],
                            start=(lkc == 0), stop=(lkc == 1),
                        )
                        if lkc == 1:
                            nc.vector.tensor_mul(aT_sb[:, seg, h, :],
                                                 avss[seg][:], invBs[seg][:])
                    return emit

                return [sc_step(0), sc_step(1),
                        sums_step(0), av_step(0, 0), av_step(0, 1),
                        sc_step(2),
                        sums_step(1), av_step(1, 0), av_step(1, 1),
                        sc_step(3),
                        sums_step(2), av_step(2, 0), av_step(2, 1),
                        sums_step(3), av_step(3, 0), av_step(3, 1)]

            def run_interleaved(gens, thunks, every):
                k, ai = 0, 0
                for g in gens:
                    for _ in g:
                        k += 1
                        if k % every == 0 and ai < len(thunks):
                            thunks[ai]()
                            ai += 1
                while ai < len(thunks):
                    thunks[ai]()
                    ai += 1

            wq_eq0 = None
            prev_qk = None
            for h in range(H):
                if h == H - 1:
                    # Prefetch the first Wout quarter one head early so the
                    # out-projection never waits on its 2MB load.
                    wq_eq0 = w_p.tile([128, DT, 512], BF, tag="wo", bufs=2,
                                      name="wq_t")
                    nc.sync.dma_start(out=wq_eq0[:], in_=wout_d[0])
                qh = qk_p.tile([128, TOK], BF, tag="qk")
                kh = qk_p.tile([128, TOK], BF, tag="qk")
                gens = [proj_steps(h, qh), proj_steps(16 + h, kh)]
                thunks = (attention_thunks(h - 1, *prev_qk)
                          if prev_qk is not None else [])
                run_interleaved(gens, thunks, every=2)
                prev_qk = (qh, kh)
            last_attn = attention_thunks(H - 1, *prev_qk)

            # ---- output projection (token-major) ----
            # Wout streamed in four 2MB e-quarters; head 15's attention
            # steps interleave into the first psum group (its aT d-tile is
            # the last accumulated, so each segment's normalize only has to
            # beat d-tile 15 of its own token tile).
            def po_steps(eq, lc, wq_t):
                seg, lqc = lc // 2, lc % 2
                po = pp_p.tile([128, 512], F32, tag="pp", name="po")
                for dt in range(DT):
                    nc.tensor.matmul(
                        po[:],
                        aT_sb[:, seg, dt, ts(lqc, 128)],
                        wq_t[:, dt, :],
                        start=(dt == 0),
                        stop=(dt == DT - 1),
                    )
                    yield
                ob = ou_p.tile([128, 512], F32, tag="ou")
                if lc % 2:
                    nc.vector.tensor_copy(out=ob[:], in_=po[:])
                else:
                    nc.scalar.activation(out=ob[:], in_=po[:], func=IDENT,
                                         scale=1.0)
                nc.sync.dma_start(
                    out=out_d[lc * 128:(lc + 1) * 128,
                              eq * 512:(eq + 1) * 512],
                    in_=ob[:],
                )

            for eq in range(4):
                if eq == 0:
                    wq_t = wq_eq0
                else:
                    wq_t = w_p.tile([128, DT, 512], BF, tag="wo", bufs=2,
                                    name="wq_t")
                    nc.sync.dma_start(out=wq_t[:], in_=wout_d[eq])
                for lc in range(TOK // 128):
                    if eq == 0 and lc < 2:
                        thunks = last_attn[8 * lc:8 * (lc + 1)]
                    else:
                        thunks = []
                    run_interleaved([po_steps(eq, lc, wq_t)], thunks,
                                    every=2)

    nc.compile()
    _dedupe_ldweights(nc)
    return nc


def _dedupe_ldweights(nc):
    """Drop InstLdweights whose weights are already resident in the PE array.

    tile_legalize emits one LDWEIGHTS per matmul; consecutive matmuls that
    share the stationary operand (projection token-halves) reload identical
    weights, costing ~97ns of PE pipe each.  Walk each block's PE stream
    tracking the loaded-weights key and delete reloads.  Only semaphore-free
    LDWEIGHTS are dropped, so the sync graph is untouched;
    EVENT_SEMAPHORE/DRAIN between pairs don't disturb the array, any other
    PE instruction conservatively invalidates the key.
    """
    from concourse import mybir

    PE = mybir.EngineType.PE
    dropped = 0
    for f in nc.m.functions:
        for blk in f.blocks:
            insts = blk.instructions
            loaded = None
            to_drop = []
            for idx, x in enumerate(insts):
                if getattr(x, "engine", None) != PE:
                    continue
                nm = type(x).__name__
                if nm == "InstLdweights":
                    si = x.sync_info
                    clean = si is None or (not si.on_wait and not si.on_update)
                    key = (str(x.ins[0]), str(x.is_transpose),
                           str(x.perf_mode), str(x.tile_position))
                    if clean and loaded == key:
                        to_drop.append(idx)
                    else:
                        loaded = key
                elif nm == "InstMatmult":
                    continue
                elif nm in ("InstEventSemaphore", "InstDrain"):
                    continue
                else:
                    loaded = None
            for idx in reversed(to_drop):
                del insts[idx]
            blk.instructions = insts
            dropped += len(to_drop)
    return dropped


def get_program():
    global _PROGRAM
    if _PROGRAM is None:
        _PROGRAM = _build_program()
    return _PROGRAM


def make_in_maps(x, Wqkv, b_qkv):
    """Host-side shard + layout prep (bf16 casts, transposes, tiling)."""
    bf16 = ml_dtypes.bfloat16
    x = np.asarray(x, dtype=np.float32)
    Wqkv = np.asarray(Wqkv, dtype=np.float32)
    b_qkv = np.asarray(b_qkv, dtype=np.float32)

    xs = x.reshape(B, NSEG, SEGMENT, D)[:, :, ::DIL, :]     # [2,16,256,2048]
    xs_flat = xs.reshape(PAIRS, L, D)

    # q/k lhsT tiles packed partition-major: wt[c, p, dt*128+j] =
    # WqkvT[dt*128+p, c*128+j] so one chunk is a single linear DMA.
    wt = np.ascontiguousarray(
        Wqkv[:2 * D].reshape(NQK, 128, DT, 128).transpose(0, 3, 2, 1)
        .reshape(NQK, 128, DT * 128)
    ).astype(bf16)                                          # [32,128,2048]
    # v moving quarters, same layout as the Wout quarters.
    wvt = _quarter_tiles(Wqkv[2 * D:])                      # [4,128,DT*512]
    bqt = np.ascontiguousarray(b_qkv[:2 * D].reshape(NQK, 128).T)  # [128,32]

    in_maps = []
    for i in range(N_CORES):
        tok = xs_flat[SPC * i:SPC * (i + 1)].reshape(TOK, D)
        xst = np.ascontiguousarray(
            tok.T.reshape(DT, 128, TOK).transpose(1, 0, 2)
            .reshape(128, DT * TOK)).astype(bf16)
        in_maps.append({"xst": xst, "wqkv_t": wt, "wv_t": wvt, "bq_t": bqt})
    return in_maps


def _quarter_tiles(W):
    """[2048, 2048] row-major -> [eq, p, dt*512+j] = W[eq*512+j, dt*128+p]."""
    W = np.asarray(W, dtype=np.float32)
    return np.ascontiguousarray(
        W.T.reshape(DT, 128, 4, 512).transpose(2, 1, 0, 3)
        .reshape(4, 128, DT * 512)).astype(ml_dtypes.bfloat16)


def make_wout_tiled(Wout):
    return _quarter_tiles(Wout)


def kernel(x, Wqkv, b_qkv, Wout, b_out):
    from concourse import bass_utils

    nc = get_program()
    in_maps = make_in_maps(x, Wqkv, b_qkv)
    wot = make_wout_tiled(Wout)
    for m in in_maps:
        m["wout_t"] = wot

    res = bass_utils.run_bass_kernel_spmd(
        nc, in_maps, core_ids=list(range(N_CORES)))
    outs = [res.results[i]["out"] for i in range(N_CORES)]
    # Softmax weights sum to 1, so the v bias passes through attention
    # unchanged: out += Wout @ b_v + b_out (host-side, exact).
    bias = (np.asarray(Wout, dtype=np.float32)
            @ np.asarray(b_qkv, dtype=np.float32)[2 * D:]
            + np.asarray(b_out, dtype=np.float32))
    full = np.concatenate(outs, axis=0) + bias
    return np.ascontiguousarray(full.reshape(B, NSEG * L, D), dtype=np.float32)


# revision 22
# speedup vs baseline: 1.0300x; 1.0275x over previous
"""Dilated segment attention on 8 TRN2 NeuronCores (Bass/Tile).

Problem (hardcoded from spec):
  x [2, 8192, 2048] f32, Wqkv [6144, 2048], b_qkv [6144], Wout [2048, 2048],
  b_out [2048].  segment=512, dilation=2 -> 16 segments of L=256 dilated
  tokens per batch; per-segment 16-head attention (hd=128); fused qkv and
  out projections.  Output [2, 4096, 2048] f32.

Sharding: the 32 (batch, segment) instances are independent -> 4 per core.
Host pre-gathers the dilated tokens, pre-transposes/pre-tiles operands and
casts to bf16 (compute precision; measured end-to-end rel err ~5e-3).

Per-core dataflow (all matmuls K=128, bf16):
  q/k proj   : feature-major  qkT[e, tok] = W-tile.T @ xsT-tile (accum 16 d-tiles)
  v proj     : token-major    v[tok, e]   = xsT-tile.T @ WvT-quarter
               (drains land directly in the AV-stationary layout — no
               transposes; DMA transposes act as global DMA barriers in
               Tile's deadlock-avoidance and measurably stall the PE)
  scores     : scoresT[lk, lq] = kT.T @ qT  (per seg, head; operands swapped)
  softmax    : exp on ScalarE (scale=1/sqrt(hd); scores provably in [-6, 6]
               so no max subtraction), sums via ones-matmul, normalize on DVE
  AV         : outT[hd, lq] = v[lk, hd].T @ expT[lk, lq]
  out proj   : out[l, e] = aT-tile.T @ WoutT-tile  (accum 16 head-tiles,
               token-major, so the HBM store is linear)
b_qkv's q/k parts are applied on-chip (they feed the softmax); its v part
and b_out are applied on the host: softmax weights sum to 1, so a v bias
shifts the attention output by exactly b_v, i.e. out += Wout @ b_v + b_out.

Schedule (v3 — trace-driven):
  - ~12 zero matmuls at kernel start (overlapping the input DMA wait) keep
    the PE HAM busy-window warm so real matmuls run at 2.4GHz immediately.
  - All startup loads on one HWDGE ring, interleaved [wv0/4, xq0, wv0/4,
    xq1, ...] so the first v-projection group ramps with the arrivals.
  - psum->sbuf drains are split/alternated ScalarE / DVE so chunk-boundary
    psum WAR frees faster and neither engine's FIFO gates the PE.
  - attention for head h-1 is interleaved between head h's projection
    matmuls (1 step per 3 d-tiles): the exp ACT latency and the per-tile
    LDWEIGHTS hide under 216ns projection matmuls.  Head 15's attention
    interleaves into the first out-projection psum groups.
"""

import numpy as np
import ml_dtypes

B = 2
S = 8192
D = 2048
H = 16
HD = 128
SEGMENT = 512
DIL = 2
NSEG = S // SEGMENT          # 16
L = SEGMENT // DIL           # 256 dilated tokens per segment
N_CORES = 8
PAIRS = B * NSEG             # 32 independent (b, n) instances
SPC = PAIRS // N_CORES       # 4 segments per core
TOK = SPC * L                # 1024 tokens per core
DT = D // 128                # 16 contraction tiles
NQK = 2 * D // 128           # 32 q/k feature chunks (16 q, 16 k)
SCALE = 1.0 / float(np.sqrt(HD))

_PROGRAM = None


def _build_program():
    import concourse.bass as bass
    import concourse.bacc as bacc
    import concourse.tile as tile
    from concourse import mybir

    BF = mybir.dt.bfloat16
    F32 = mybir.dt.float32
    ts = bass.ts
    IDENT = mybir.ActivationFunctionType.Identity
    EXP = mybir.ActivationFunctionType.Exp

    nc = bacc.Bacc("TRN2", target_bir_lowering=False, debug=False,
                   num_devices=N_CORES)

    xst_d = nc.dram_tensor("xst", [128, DT * TOK], BF, kind="ExternalInput")
    wqkv_d = nc.dram_tensor("wqkv_t", [NQK, 128, DT * 128], BF,
                            kind="ExternalInput")
    wv_d = nc.dram_tensor("wv_t", [4, 128, DT * 512], BF, kind="ExternalInput")
    wout_d = nc.dram_tensor("wout_t", [4, 128, DT * 512], BF, kind="ExternalInput")
    bq_d = nc.dram_tensor("bq_t", [128, NQK], F32, kind="ExternalInput")
    out_d = nc.dram_tensor("out", [TOK, D], F32, kind="ExternalOutput")

    with tile.TileContext(nc) as tc:
        with (
            tc.tile_pool(name="const", bufs=1) as const_p,
            tc.tile_pool(name="big", bufs=1) as big_p,
            tc.tile_pool(name="wq", bufs=6) as w_p,
            tc.tile_pool(name="qk", bufs=4) as qk_p,
            tc.tile_pool(name="ex", bufs=4) as ex_p,
            tc.tile_pool(name="st", bufs=2) as st_p,
            tc.tile_pool(name="ou", bufs=3) as ou_p,
            tc.tile_pool(name="pp", bufs=4, space="PSUM") as pp_p,
            tc.tile_pool(name="pa", bufs=2, space="PSUM") as pa_p,
        ):
            # All-ones stationary for the softmax sums: a [128, 128] ones
            # matrix replicates the column sums across all 128 output
            # partitions, so the reciprocal needs no partition_broadcast
            # and the matmul output drains full-width (1-row outputs incur
            # a serial-drain penalty, measured +100ns on the matmul AND on
            # the instruction after it).
            ones = const_p.tile([128, 128], BF)
            nc.gpsimd.memset(ones[:], 1.0)
            # PE warmup: the HAM clock gate holds the PE at 1.2GHz until
            # ~3.4us of sustained activity.  Burn that window on zero
            # matmuls while the input DMAs stream, so the first real
            # matmul runs at 2.4GHz.
            warm_sb = const_p.tile([128, 512], BF)
            nc.gpsimd.memset(warm_sb[:], 0.0)
            warm_ps = pp_p.tile([128, 512], F32, tag="pp", name="warm")
            for i in range(16):
                nc.tensor.matmul(warm_ps[:], warm_sb[:, 0:128], warm_sb[:],
                                 start=(i == 0), stop=(i == 15))

            # Startup loads: one FIFO ring, v-quarter-0 slices interleaved
            # with the xst quarters so the first v group's d-tile matmuls
            # start as each (weights, activations) pair lands.
            wv_tiles = [None] * 4
            wv_tiles[0] = w_p.tile([128, DT, 512], BF, tag="wo", bufs=2,
                                   name="wv_t")
            xq_sb = [big_p.tile([128, 4, TOK], BF, name=f"xq{k}")
                     for k in range(4)]
            bq_sb = const_p.tile([128, NQK], F32)
            nc.scalar.dma_start(out=bq_sb[:], in_=bq_d[:])
            for k in range(4):
                nc.sync.dma_start(out=wv_tiles[0][:, 4 * k:4 * (k + 1), :],
                                  in_=wv_d[0][:, 4 * k * 512:4 * (k + 1) * 512])
                nc.sync.dma_start(out=xq_sb[k][:],
                                  in_=xst_d[:, 4 * k * TOK:4 * (k + 1) * TOK])
            vtok_sb = big_p.tile([128, H, SPC * 2, 128], BF)
            aT_sb = big_p.tile([128, SPC, H, L], BF)

            # ---- v projection, token-major (out-proj-style groups) ----
            # vtok[p, h, t, j] = v[tok = t*128+p, hd = j]: each (vc, t)
            # group accumulates [128 tok, 512 v-feats] over the 16 d-tiles
            # and drains straight into the AV-stationary layout.
            def v_drain(vc, t, vps):
                dst = vtok_sb[:, 4 * vc:4 * (vc + 1), t, :]
                if t % 2:
                    nc.vector.tensor_copy(out=dst, in_=vps[:])
                else:
                    nc.scalar.activation(out=dst, in_=vps[:], func=IDENT,
                                         scale=1.0)

            # Double-buffered wv prefetch: vc+1's 2MB load is dispatched at
            # the start of vc's compute.
            wv_tiles[1] = w_p.tile([128, DT, 512], BF, tag="wo", bufs=2,
                                   name="wv_t")
            nc.sync.dma_start(out=wv_tiles[1][:], in_=wv_d[1])

            # vc0 rides the input-DMA ramp: sweep d-tile-quarter-major
            # across 4 concurrent token-tile psum groups, so every quarter
            # arrival unlocks 16 matmuls instead of 4 (the in-order PE
            # stream would otherwise head-of-line block on group 0's next
            # quarter while groups 1-7's early d-tiles are ready).
            for wave in range(2):
                vpss = [pp_p.tile([128, 512], F32, tag="pp", name="vps")
                        for _ in range(4)]
                for qk in range(4):
                    for ti in range(4):
                        for r in range(4):
                            nc.tensor.matmul(
                                vpss[ti][:],
                                xq_sb[qk][:, r, (4 * wave + ti) * 128:
                                          (4 * wave + ti + 1) * 128],
                                wv_tiles[0][:, 4 * qk + r, :],
                                start=(qk == 0 and r == 0),
                                stop=(qk == 3 and r == 3),
                            )
                for ti in range(4):
                    v_drain(0, 4 * wave + ti, vpss[ti])

            for vc in range(1, 4):
                if vc + 1 < 4:
                    wv_tiles[vc + 1] = w_p.tile([128, DT, 512], BF, tag="wo",
                                                bufs=2, name="wv_t")
                    nc.sync.dma_start(out=wv_tiles[vc + 1][:],
                                      in_=wv_d[vc + 1])
                for t in range(SPC * 2):
                    vps = pp_p.tile([128, 512], F32, tag="pp", name="vps")
                    for dt in range(DT):
                        q, r = divmod(dt, 4)
                        nc.tensor.matmul(
                            vps[:],
                            xq_sb[q][:, r, t * 128:(t + 1) * 128],
                            wv_tiles[vc][:, dt, :],
                            start=(dt == 0),
                            stop=(dt == DT - 1),
                        )
                    v_drain(vc, t, vps)

            def proj_steps(c, out_tile):
                """q/k chunk c: out_tile[128, TOK] bf16 = (W chunk).T @ xsT + b.

                Generator: yields after each d-tile's matmul pair; emits the
                split-engine psum drains on exhaustion.
                """
                wck = w_p.tile([128, DT * 128], BF, tag="w")
                nc.sync.dma_start(out=wck[:], in_=wqkv_d[c])
                pss = [pp_p.tile([128, 512], F32, tag="pp", name=f"ps{half}")
                       for half in range(2)]
                for dt in range(DT):
                    q, r = divmod(dt, 4)
                    for half in range(2):
                        nc.tensor.matmul(
                            pss[half][:],
                            wck[:, ts(dt, 128)],
                            xq_sb[q][:, r, ts(half, 512)],
                            start=(dt == 0),
                            stop=(dt == DT - 1),
                        )
                    yield
                nc.scalar.activation(
                    out=out_tile[:, ts(0, 512)], in_=pss[0][:],
                    func=IDENT, bias=bq_sb[:, c:c + 1], scale=1.0,
                )
                nc.vector.tensor_scalar_add(
                    out_tile[:, ts(1, 512)], pss[1][:], bq_sb[:, c:c + 1],
                )

            # ---- per-head attention, interleaved into the next head's
            # projection matmul stream ----
            def attention_thunks(h, qh, kh):
                """16 emission steps for head h's attention over 4 segments.

                Each step issues at most ONE new PE stationary (scores pair
                shares the q moving operand; sums pair shares the ones
                stationary after LDWEIGHTS dedupe), so with >=2 projection
                matmuls between steps every attention LDWEIGHTS loads into
                the background weight buffer under a 216ns projection
                matmul instead of stalling the PE (measured +90..160ns on
                every av-group matmul when sums+AV+AV issued back-to-back).
                """
                e_ts = [None] * SPC
                scts = [None] * SPC
                avss = [None] * SPC

                def sc_step(seg):
                    def emit():
                        scT = scts[seg] = pa_p.tile([128, 2, L], F32,
                                                    tag="pa", name="scT")
                        for lkc in range(2):
                            nc.tensor.matmul(
                                scT[:, lkc, :],
                                kh[:, seg * L + lkc * 128:
                                   seg * L + (lkc + 1) * 128],
                                qh[:, seg * L:(seg + 1) * L],
                            )
                        e_t = ex_p.tile([128, 2, L], BF, tag="ex")
                        nc.scalar.activation(out=e_t[:], in_=scT[:],
                                             func=EXP, scale=SCALE)
                        e_ts[seg] = e_t
                    return emit

                def sums_step(seg):
                    def emit():
                        # av ([:, 0, :]) and the replicated softmax sums
                        # ([:, 1, :]) share one PSUM bank; Tile serializes
                        # the DVE reads behind all four matmul writes, which
                        # the mul needs anyway.
                        avs = avss[seg] = pa_p.tile([128, 2, L], F32,
                                                    tag="pav", bufs=2,
                                                    name="avs")
                        for lkc in range(2):
                            nc.tensor.matmul(
                                avs[:, 1, :], ones[:], e_ts[seg][:, lkc, :],
                                start=(lkc == 0), stop=(lkc == 1),
                            )
                    return emit

                def av_step(seg, lkc):
                    def emit():
                        avs = avss[seg]
                        nc.tensor.matmul(
                            avs[:, 0, :],
                            vtok_sb[:, h, seg * 2 + lkc, :],
                            e_ts[seg][:, lkc, :],
                            start=(lkc == 0), stop=(lkc == 1),
                        )
                        if lkc == 1:
                            invB = ex_p.tile([128, L], F32, tag="invb",
                                             name="invB")
                            nc.vector.reciprocal_approx_fast(
                                out=invB[:], in_=avs[:, 1, :])
                            nc.vector.tensor_mul(aT_sb[:, seg, h, :],
                                                 avs[:, 0, :], invB[:])
                    return emit

                return [sc_step(0), sc_step(1), sc_step(2),
                        sums_step(0), av_step(0, 0), av_step(0, 1),
                        sums_step(1), av_step(1, 0), av_step(1, 1),
                        sc_step(3),
                        sums_step(2), av_step(2, 0), av_step(2, 1),
                        sums_step(3), av_step(3, 0), av_step(3, 1)]

            def run_interleaved(gens, thunks, every):
                k, ai = 0, 0
                for g in gens:
                    for _ in g:
                        k += 1
                        if k % every == 0 and ai < len(thunks):
                            thunks[ai]()
                            ai += 1
                while ai < len(thunks):
                    thunks[ai]()
                    ai += 1

            wq_eq0 = None
            prev_qk = None
            for h in range(H):
                if h == H - 1:
                    # Prefetch the first Wout quarter one head early so the
                    # out-projection never waits on its 2MB load.
                    wq_eq0 = w_p.tile([128, DT, 512], BF, tag="wo", bufs=2,
                                      name="wq_t")
                    nc.sync.dma_start(out=wq_eq0[:], in_=wout_d[0])
                qh = qk_p.tile([128, TOK], BF, tag="qk")
                kh = qk_p.tile([128, TOK], BF, tag="qk")
                gens = [proj_steps(h, qh), proj_steps(16 + h, kh)]
                thunks = (attention_thunks(h - 1, *prev_qk)
                          if prev_qk is not None else [])
                run_interleaved(gens, thunks, every=2)
                prev_qk = (qh, kh)
            last_attn = attention_thunks(H - 1, *prev_qk)

            # ---- output projection (token-major) ----
            # Wout streamed in four 2MB e-quarters; head 15's attention
            # steps interleave into the first psum group (its aT d-tile is
            # the last accumulated, so each segment's normalize only has to
            # beat d-tile 15 of its own token tile).
            def po_steps(eq, lc, wq_t):
                seg, lqc = lc // 2, lc % 2
                po = pp_p.tile([128, 512], F32, tag="pp", name="po")
                for dt in range(DT):
                    nc.tensor.matmul(
                        po[:],
                        aT_sb[:, seg, dt, ts(lqc, 128)],
                        wq_t[:, dt, :],
                        start=(dt == 0),
                        stop=(dt == DT - 1),
                    )
                    yield
                ob = ou_p.tile([128, 512], F32, tag="ou")
                if lc % 2:
                    nc.vector.tensor_copy(out=ob[:], in_=po[:])
                else:
                    nc.scalar.activation(out=ob[:], in_=po[:], func=IDENT,
                                         scale=1.0)
                nc.sync.dma_start(
                    out=out_d[lc * 128:(lc + 1) * 128,
                              eq * 512:(eq + 1) * 512],
                    in_=ob[:],
                )

            for eq in range(4):
                if eq == 0:
                    wq_t = wq_eq0
                else:
                    wq_t = w_p.tile([128, DT, 512], BF, tag="wo", bufs=2,
                                    name="wq_t")
                    nc.sync.dma_start(out=wq_t[:], in_=wout_d[eq])
                for lc in range(TOK // 128):
                    if eq == 0 and lc < 2:
                        thunks = last_attn[8 * lc:8 * (lc + 1)]
                    else:
                        thunks = []
                    run_interleaved([po_steps(eq, lc, wq_t)], thunks,
                                    every=2)

    nc.compile()
    _dedupe_ldweights(nc)
    return nc


def _dedupe_ldweights(nc):
    """Drop InstLdweights whose weights are already resident in the PE array.

    tile_legalize emits one LDWEIGHTS per matmul; consecutive matmuls that
    share the stationary operand (projection token-halves) reload identical
    weights, costing ~97ns of PE pipe each.  Walk each block's PE stream
    tracking the loaded-weights key and delete reloads.  Only semaphore-free
    LDWEIGHTS are dropped, so the sync graph is untouched;
    EVENT_SEMAPHORE/DRAIN between pairs don't disturb the array, any other
    PE instruction conservatively invalidates the key.
    """
    from concourse import mybir

    PE = mybir.EngineType.PE
    dropped = 0
    for f in nc.m.functions:
        for blk in f.blocks:
            insts = blk.instructions
            loaded = None
            to_drop = []
            for idx, x in enumerate(insts):
                if getattr(x, "engine", None) != PE:
                    continue
                nm = type(x).__name__
                if nm == "InstLdweights":
                    si = x.sync_info
                    clean = si is None or (not si.on_wait and not si.on_update)
                    key = (str(x.ins[0]), str(x.is_transpose),
                           str(x.perf_mode), str(x.tile_position))
                    if clean and loaded == key:
                        to_drop.append(idx)
                    else:
                        loaded = key
                elif nm == "InstMatmult":
                    continue
                elif nm in ("InstEventSemaphore", "InstDrain"):
                    continue
                else:
                    loaded = None
            for idx in reversed(to_drop):
                del insts[idx]
            blk.instructions = insts
            dropped += len(to_drop)
    return dropped


def get_program():
    global _PROGRAM
    if _PROGRAM is None:
        _PROGRAM = _build_program()
    return _PROGRAM


def make_in_maps(x, Wqkv, b_qkv):
    """Host-side shard + layout prep (bf16 casts, transposes, tiling)."""
    bf16 = ml_dtypes.bfloat16
    x = np.asarray(x, dtype=np.float32)
    Wqkv = np.asarray(Wqkv, dtype=np.float32)
    b_qkv = np.asarray(b_qkv, dtype=np.float32)

    xs = x.reshape(B, NSEG, SEGMENT, D)[:, :, ::DIL, :]     # [2,16,256,2048]
    xs_flat = xs.reshape(PAIRS, L, D)

    # q/k lhsT tiles packed partition-major: wt[c, p, dt*128+j] =
    # WqkvT[dt*128+p, c*128+j] so one chunk is a single linear DMA.
    wt = np.ascontiguousarray(
        Wqkv[:2 * D].reshape(NQK, 128, DT, 128).transpose(0, 3, 2, 1)
        .reshape(NQK, 128, DT * 128)
    ).astype(bf16)                                          # [32,128,2048]
    # v moving quarters, same layout as the Wout quarters.
    wvt = _quarter_tiles(Wqkv[2 * D:])                      # [4,128,DT*512]
    bqt = np.ascontiguousarray(b_qkv[:2 * D].reshape(NQK, 128).T)  # [128,32]

    in_maps = []
    for i in range(N_CORES):
        tok = xs_flat[SPC * i:SPC * (i + 1)].reshape(TOK, D)
        xst = np.ascontiguousarray(
            tok.T.reshape(DT, 128, TOK).transpose(1, 0, 2)
            .reshape(128, DT * TOK)).astype(bf16)
        in_maps.append({"xst": xst, "wqkv_t": wt, "wv_t": wvt, "bq_t": bqt})
    return in_maps


def _quarter_tiles(W):
    """[2048, 2048] row-major -> [eq, p, dt*512+j] = W[eq*512+j, dt*128+p]."""
    W = np.asarray(W, dtype=np.float32)
    return np.ascontiguousarray(
        W.T.reshape(DT, 128, 4, 512).transpose(2, 1, 0, 3)
        .reshape(4, 128, DT * 512)).astype(ml_dtypes.bfloat16)


def make_wout_tiled(Wout):
    return _quarter_tiles(Wout)


def kernel(x, Wqkv, b_qkv, Wout, b_out):
    from concourse import bass_utils

    nc = get_program()
    in_maps = make_in_maps(x, Wqkv, b_qkv)
    wot = make_wout_tiled(Wout)
    for m in in_maps:
        m["wout_t"] = wot

    res = bass_utils.run_bass_kernel_spmd(
        nc, in_maps, core_ids=list(range(N_CORES)))
    outs = [res.results[i]["out"] for i in range(N_CORES)]
    # Softmax weights sum to 1, so the v bias passes through attention
    # unchanged: out += Wout @ b_v + b_out (host-side, exact).
    bias = (np.asarray(Wout, dtype=np.float32)
            @ np.asarray(b_qkv, dtype=np.float32)[2 * D:]
            + np.asarray(b_out, dtype=np.float32))
    full = np.concatenate(outs, axis=0) + bias
    return np.ascontiguousarray(full.reshape(B, NSEG * L, D), dtype=np.float32)


# revision 30
# speedup vs baseline: 1.0304x; 1.0004x over previous
"""Dilated segment attention on 8 TRN2 NeuronCores (Bass/Tile).

Problem (hardcoded from spec):
  x [2, 8192, 2048] f32, Wqkv [6144, 2048], b_qkv [6144], Wout [2048, 2048],
  b_out [2048].  segment=512, dilation=2 -> 16 segments of L=256 dilated
  tokens per batch; per-segment 16-head attention (hd=128); fused qkv and
  out projections.  Output [2, 4096, 2048] f32.

Sharding: the 32 (batch, segment) instances are independent -> 4 per core.
Host pre-gathers the dilated tokens, pre-transposes/pre-tiles operands and
casts to bf16 (compute precision; measured end-to-end rel err ~5e-3).

Per-core dataflow (all matmuls K=128, bf16):
  q/k proj   : feature-major  qkT[e, tok] = W-tile.T @ xsT-tile (accum 16 d-tiles)
  v proj     : token-major    v[tok, e]   = xsT-tile.T @ WvT-quarter
               (drains land directly in the AV-stationary layout — no
               transposes; DMA transposes act as global DMA barriers in
               Tile's deadlock-avoidance and measurably stall the PE)
  scores     : scoresT[lk, lq] = kT.T @ qT  (per seg, head; operands swapped)
  softmax    : exp on ScalarE (scale=1/sqrt(hd); scores provably in [-6, 6]
               so no max subtraction), sums via ones-matmul, normalize on DVE
  AV         : outT[hd, lq] = v[lk, hd].T @ expT[lk, lq]
  out proj   : out[l, e] = aT-tile.T @ WoutT-tile  (accum 16 head-tiles,
               token-major, so the HBM store is linear)
b_qkv's q/k parts are applied on-chip (they feed the softmax); its v part
and b_out are applied on the host: softmax weights sum to 1, so a v bias
shifts the attention output by exactly b_v, i.e. out += Wout @ b_v + b_out.

Schedule (v3 — trace-driven):
  - ~12 zero matmuls at kernel start (overlapping the input DMA wait) keep
    the PE HAM busy-window warm so real matmuls run at 2.4GHz immediately.
  - All startup loads on one HWDGE ring, interleaved [wv0/4, xq0, wv0/4,
    xq1, ...] so the first v-projection group ramps with the arrivals.
  - psum->sbuf drains are split/alternated ScalarE / DVE so chunk-boundary
    psum WAR frees faster and neither engine's FIFO gates the PE.
  - attention for head h-1 is interleaved between head h's projection
    matmuls (1 step per 3 d-tiles): the exp ACT latency and the per-tile
    LDWEIGHTS hide under 216ns projection matmuls.  Head 15's attention
    interleaves into the first out-projection psum groups.
"""

import numpy as np
import ml_dtypes

B = 2
S = 8192
D = 2048
H = 16
HD = 128
SEGMENT = 512
DIL = 2
NSEG = S // SEGMENT          # 16
L = SEGMENT // DIL           # 256 dilated tokens per segment
N_CORES = 8
PAIRS = B * NSEG             # 32 independent (b, n) instances
SPC = PAIRS // N_CORES       # 4 segments per core
TOK = SPC * L                # 1024 tokens per core
DT = D // 128                # 16 contraction tiles
NQK = 2 * D // 128           # 32 q/k feature chunks (16 q, 16 k)
SCALE = 1.0 / float(np.sqrt(HD))

_PROGRAM = None


def _build_program():
    import concourse.bass as bass
    import concourse.bacc as bacc
    import concourse.tile as tile
    from concourse import mybir

    BF = mybir.dt.bfloat16
    F32 = mybir.dt.float32
    ts = bass.ts
    IDENT = mybir.ActivationFunctionType.Identity
    EXP = mybir.ActivationFunctionType.Exp

    nc = bacc.Bacc("TRN2", target_bir_lowering=False, debug=False,
                   num_devices=N_CORES)

    xst_d = nc.dram_tensor("xst", [128, DT, TOK], BF, kind="ExternalInput")
    wqkv_d = nc.dram_tensor("wqkv_t", [NQK, 128, DT * 128], BF,
                            kind="ExternalInput")
    wv_d = nc.dram_tensor("wv_t", [4, 128, DT * 512], BF, kind="ExternalInput")
    wout_d = nc.dram_tensor("wout_t", [4, 128, DT * 512], BF, kind="ExternalInput")
    bq_d = nc.dram_tensor("bq_t", [128, NQK], F32, kind="ExternalInput")
    out_d = nc.dram_tensor("out", [TOK, D], F32, kind="ExternalOutput")

    with tile.TileContext(nc) as tc:
        with (
            tc.tile_pool(name="const", bufs=1) as const_p,
            tc.tile_pool(name="big", bufs=1) as big_p,
            tc.tile_pool(name="wq", bufs=6) as w_p,
            tc.tile_pool(name="qk", bufs=4) as qk_p,
            tc.tile_pool(name="ex", bufs=4) as ex_p,
            tc.tile_pool(name="st", bufs=2) as st_p,
            tc.tile_pool(name="ou", bufs=3) as ou_p,
            tc.tile_pool(name="pp", bufs=4, space="PSUM") as pp_p,
            tc.tile_pool(name="pa", bufs=2, space="PSUM") as pa_p,
        ):
            # All-ones stationary for the softmax sums: a [128, 128] ones
            # matrix replicates the column sums across all 128 output
            # partitions, so the reciprocal needs no partition_broadcast
            # and the matmul output drains full-width (1-row outputs incur
            # a serial-drain penalty, measured +100ns on the matmul AND on
            # the instruction after it).
            ones = const_p.tile([128, 128], BF)
            nc.gpsimd.memset(ones[:], 1.0)
            # PE warmup: the HAM clock gate holds the PE at 1.2GHz until
            # ~3.4us of sustained activity.  Burn that window on zero
            # matmuls while the input DMAs stream, so the first real
            # matmul runs at 2.4GHz.
            warm_sb = const_p.tile([128, 512], BF)
            nc.gpsimd.memset(warm_sb[:], 0.0)
            warm_ps = pp_p.tile([128, 512], F32, tag="pp", name="warm")
            for i in range(8):
                nc.tensor.matmul(warm_ps[:], warm_sb[:, 0:128], warm_sb[:],
                                 start=(i == 0), stop=(i == 7))

            # Startup loads: one FIFO ring ordered to match the first
            # v-wave's consumption exactly — [wv0/4, xq-quarter first
            # halves]x4, then the second halves.  The first 16 matmuls need
            # only 1MB; each further d-tile quarter's (weights, tokens 0-511)
            # pair lands one compute-burst ahead of its use.
            wv_tiles = [None] * 4
            wv_tiles[0] = w_p.tile([128, DT, 512], BF, tag="wo", bufs=2,
                                   name="wv_t")
            # Token-halves are separate tiles so each consumer's dependency
            # binds to exactly one DMA (subtile tracking proved coarse).
            xh_sb = [[big_p.tile([128, 4, 512], BF, name=f"xq{k}h{hf}")
                      for hf in range(2)] for k in range(4)]
            bq_sb = const_p.tile([128, NQK], F32)
            nc.scalar.dma_start(out=bq_sb[:], in_=bq_d[:])
            for k in range(4):
                nc.sync.dma_start(out=wv_tiles[0][:, 4 * k:4 * (k + 1), :],
                                  in_=wv_d[0][:, 4 * k * 512:4 * (k + 1) * 512])
                nc.sync.dma_start(out=xh_sb[k][0][:],
                                  in_=xst_d[:, 4 * k:4 * (k + 1), 0:512])
            for k in range(4):
                nc.sync.dma_start(out=xh_sb[k][1][:],
                                  in_=xst_d[:, 4 * k:4 * (k + 1), 512:TOK])
            vtok_sb = big_p.tile([128, H, SPC * 2, 128], BF)
            aT_sb = big_p.tile([128, SPC, H, L], BF)

            # ---- v projection, token-major (out-proj-style groups) ----
            # vtok[p, h, t, j] = v[tok = t*128+p, hd = j]: each (vc, t)
            # group accumulates [128 tok, 512 v-feats] over the 16 d-tiles
            # and drains straight into the AV-stationary layout.
            def v_drain(vc, t, vps):
                dst = vtok_sb[:, 4 * vc:4 * (vc + 1), t, :]
                if t % 2:
                    nc.vector.tensor_copy(out=dst, in_=vps[:])
                else:
                    nc.scalar.activation(out=dst, in_=vps[:], func=IDENT,
                                         scale=1.0)

            # Double-buffered wv prefetch: vc+1's 2MB load is dispatched at
            # the start of vc's compute.
            wv_tiles[1] = w_p.tile([128, DT, 512], BF, tag="wo", bufs=2,
                                   name="wv_t")
            nc.sync.dma_start(out=wv_tiles[1][:], in_=wv_d[1])

            # vc0 rides the input-DMA ramp: sweep d-tile-quarter-major
            # across 4 concurrent token-tile psum groups, so every quarter
            # arrival unlocks 16 matmuls instead of 4 (the in-order PE
            # stream would otherwise head-of-line block on group 0's next
            # quarter while groups 1-7's early d-tiles are ready).
            for wave in range(2):
                vpss = [pp_p.tile([128, 512], F32, tag="pp", name="vps")
                        for _ in range(4)]
                for qk in range(4):
                    for ti in range(4):
                        for r in range(4):
                            nc.tensor.matmul(
                                vpss[ti][:],
                                xh_sb[qk][wave][:, r, ti * 128:
                                                (ti + 1) * 128],
                                wv_tiles[0][:, 4 * qk + r, :],
                                start=(qk == 0 and r == 0),
                                stop=(qk == 3 and r == 3),
                            )
                for ti in range(4):
                    v_drain(0, 4 * wave + ti, vpss[ti])

            for vc in range(1, 4):
                if vc + 1 < 4:
                    wv_tiles[vc + 1] = w_p.tile([128, DT, 512], BF, tag="wo",
                                                bufs=2, name="wv_t")
                    nc.sync.dma_start(out=wv_tiles[vc + 1][:],
                                      in_=wv_d[vc + 1])
                for t in range(SPC * 2):
                    vps = pp_p.tile([128, 512], F32, tag="pp", name="vps")
                    for dt in range(DT):
                        q, r = divmod(dt, 4)
                        nc.tensor.matmul(
                            vps[:],
                            xh_sb[q][t // 4][:, r, (t % 4) * 128:
                                             (t % 4 + 1) * 128],
                            wv_tiles[vc][:, dt, :],
                            start=(dt == 0),
                            stop=(dt == DT - 1),
                        )
                    v_drain(vc, t, vps)

            def proj_steps(c, out_tile):
                """q/k chunk c: out_tile[128, TOK] bf16 = (W chunk).T @ xsT + b.

                Generator: yields after each d-tile's matmul pair; emits the
                split-engine psum drains on exhaustion.
                """
                wck = w_p.tile([128, DT * 128], BF, tag="w")
                nc.sync.dma_start(out=wck[:], in_=wqkv_d[c])
                pss = [pp_p.tile([128, 512], F32, tag="pp", name=f"ps{half}")
                       for half in range(2)]
                for dt in range(DT):
                    q, r = divmod(dt, 4)
                    for half in range(2):
                        nc.tensor.matmul(
                            pss[half][:],
                            wck[:, ts(dt, 128)],
                            xh_sb[q][half][:, r, :],
                            start=(dt == 0),
                            stop=(dt == DT - 1),
                        )
                    yield
                nc.scalar.activation(
                    out=out_tile[:, ts(0, 512)], in_=pss[0][:],
                    func=IDENT, bias=bq_sb[:, c:c + 1], scale=1.0,
                )
                nc.vector.tensor_scalar_add(
                    out_tile[:, ts(1, 512)], pss[1][:], bq_sb[:, c:c + 1],
                )

            # ---- per-head attention, interleaved into the next head's
            # projection matmul stream ----
            def attention_thunks(h, qh, kh):
                """16 emission steps for head h's attention over 4 segments.

                Each step issues at most ONE new PE stationary (scores pair
                shares the q moving operand; sums pair shares the ones
                stationary after LDWEIGHTS dedupe), so with >=2 projection
                matmuls between steps every attention LDWEIGHTS loads into
                the background weight buffer under a 216ns projection
                matmul instead of stalling the PE (measured +90..160ns on
                every av-group matmul when sums+AV+AV issued back-to-back).
                """
                e_ts = [None] * SPC
                scts = [None] * SPC
                avss = [None] * SPC

                def sc_step(seg):
                    def emit():
                        scT = scts[seg] = pa_p.tile([128, 2, L], F32,
                                                    tag="pa", name="scT")
                        for lkc in range(2):
                            nc.tensor.matmul(
                                scT[:, lkc, :],
                                kh[:, seg * L + lkc * 128:
                                   seg * L + (lkc + 1) * 128],
                                qh[:, seg * L:(seg + 1) * L],
                            )
                        e_t = ex_p.tile([128, 2, L], BF, tag="ex")
                        nc.scalar.activation(out=e_t[:], in_=scT[:],
                                             func=EXP, scale=SCALE)
                        e_ts[seg] = e_t
                    return emit

                def sums_step(seg):
                    def emit():
                        # av ([:, 0, :]) and the replicated softmax sums
                        # ([:, 1, :]) share one PSUM bank; Tile serializes
                        # the DVE reads behind all four matmul writes, which
                        # the mul needs anyway.
                        avs = avss[seg] = pa_p.tile([128, 2, L], F32,
                                                    tag="pav", bufs=2,
                                                    name="avs")
                        for lkc in range(2):
                            nc.tensor.matmul(
                                avs[:, 1, :], ones[:], e_ts[seg][:, lkc, :],
                                start=(lkc == 0), stop=(lkc == 1),
                            )
                    return emit

                def av_step(seg, lkc):
                    def emit():
                        avs = avss[seg]
                        nc.tensor.matmul(
                            avs[:, 0, :],
                            vtok_sb[:, h, seg * 2 + lkc, :],
                            e_ts[seg][:, lkc, :],
                            start=(lkc == 0), stop=(lkc == 1),
                        )
                        if lkc == 1:
                            invB = ex_p.tile([128, L], F32, tag="invb",
                                             name="invB")
                            nc.vector.reciprocal_approx_fast(
                                out=invB[:], in_=avs[:, 1, :])
                            nc.vector.tensor_mul(aT_sb[:, seg, h, :],
                                                 avs[:, 0, :], invB[:])
                    return emit

                return [sc_step(0), sc_step(1), sc_step(2),
                        sums_step(0), av_step(0, 0), av_step(0, 1),
                        sums_step(1), av_step(1, 0), av_step(1, 1),
                        sc_step(3),
                        sums_step(2), av_step(2, 0), av_step(2, 1),
                        sums_step(3), av_step(3, 0), av_step(3, 1)]

            def run_interleaved(gens, thunks, every):
                k, ai = 0, 0
                for g in gens:
                    for _ in g:
                        k += 1
                        if k % every == 0 and ai < len(thunks):
                            thunks[ai]()
                            ai += 1
                while ai < len(thunks):
                    thunks[ai]()
                    ai += 1

            wq_eq0 = None
            prev_qk = None
            for h in range(H):
                if h == H - 1:
                    # Prefetch the first Wout quarter one head early so the
                    # out-projection never waits on its 2MB load.
                    wq_eq0 = w_p.tile([128, DT, 512], BF, tag="wo", bufs=2,
                                      name="wq_t")
                    nc.sync.dma_start(out=wq_eq0[:], in_=wout_d[0])
                qh = qk_p.tile([128, TOK], BF, tag="qk")
                kh = qk_p.tile([128, TOK], BF, tag="qk")
                gens = [proj_steps(h, qh), proj_steps(16 + h, kh)]
                thunks = (attention_thunks(h - 1, *prev_qk)
                          if prev_qk is not None else [])
                run_interleaved(gens, thunks, every=2)
                prev_qk = (qh, kh)
            last_attn = attention_thunks(H - 1, *prev_qk)

            # ---- output projection (token-major) ----
            # Wout streamed in four 2MB e-quarters; head 15's attention
            # steps interleave into the first psum group (its aT d-tile is
            # the last accumulated, so each segment's normalize only has to
            # beat d-tile 15 of its own token tile).
            def po_steps(eq, lc, wq_t):
                seg, lqc = lc // 2, lc % 2
                po = pp_p.tile([128, 512], F32, tag="pp", name="po")
                for dt in range(DT):
                    nc.tensor.matmul(
                        po[:],
                        aT_sb[:, seg, dt, ts(lqc, 128)],
                        wq_t[:, dt, :],
                        start=(dt == 0),
                        stop=(dt == DT - 1),
                    )
                    yield
                ob = ou_p.tile([128, 512], F32, tag="ou")
                if lc % 2:
                    nc.vector.tensor_copy(out=ob[:], in_=po[:])
                else:
                    nc.scalar.activation(out=ob[:], in_=po[:], func=IDENT,
                                         scale=1.0)
                nc.sync.dma_start(
                    out=out_d[lc * 128:(lc + 1) * 128,
                              eq * 512:(eq + 1) * 512],
                    in_=ob[:],
                )

            for eq in range(4):
                if eq == 0:
                    wq_t = wq_eq0
                else:
                    wq_t = w_p.tile([128, DT, 512], BF, tag="wo", bufs=2,
                                    name="wq_t")
                    nc.sync.dma_start(out=wq_t[:], in_=wout_d[eq])
                for lc in range(TOK // 128):
                    if eq == 0 and lc < 2:
                        thunks = last_attn[8 * lc:8 * (lc + 1)]
                    else:
                        thunks = []
                    run_interleaved([po_steps(eq, lc, wq_t)], thunks,
                                    every=2)

    nc.compile()
    _dedupe_ldweights(nc)
    return nc


def _dedupe_ldweights(nc):
    """Drop InstLdweights whose weights are already resident in the PE array.

    tile_legalize emits one LDWEIGHTS per matmul; consecutive matmuls that
    share the stationary operand (projection token-halves) reload identical
    weights, costing ~97ns of PE pipe each.  Walk each block's PE stream
    tracking the loaded-weights key and delete reloads.  Only semaphore-free
    LDWEIGHTS are dropped, so the sync graph is untouched;
    EVENT_SEMAPHORE/DRAIN between pairs don't disturb the array, any other
    PE instruction conservatively invalidates the key.
    """
    from concourse import mybir

    PE = mybir.EngineType.PE
    dropped = 0
    for f in nc.m.functions:
        for blk in f.blocks:
            insts = blk.instructions
            loaded = None
            to_drop = []
            for idx, x in enumerate(insts):
                if getattr(x, "engine", None) != PE:
                    continue
                nm = type(x).__name__
                if nm == "InstLdweights":
                    si = x.sync_info
                    clean = si is None or (not si.on_wait and not si.on_update)
                    key = (str(x.ins[0]), str(x.is_transpose),
                           str(x.perf_mode), str(x.tile_position))
                    if clean and loaded == key:
                        to_drop.append(idx)
                    else:
                        loaded = key
                elif nm == "InstMatmult":
                    continue
                elif nm in ("InstEventSemaphore", "InstDrain"):
                    continue
                else:
                    loaded = None
            for idx in reversed(to_drop):
                del insts[idx]
            blk.instructions = insts
            dropped += len(to_drop)
    return dropped


def get_program():
    global _PROGRAM
    if _PROGRAM is None:
        _PROGRAM = _build_program()
    return _PROGRAM


def make_in_maps(x, Wqkv, b_qkv):
    """Host-side shard + layout prep (bf16 casts, transposes, tiling)."""
    bf16 = ml_dtypes.bfloat16
    x = np.asarray(x, dtype=np.float32)
    Wqkv = np.asarray(Wqkv, dtype=np.float32)
    b_qkv = np.asarray(b_qkv, dtype=np.float32)

    xs = x.reshape(B, NSEG, SEGMENT, D)[:, :, ::DIL, :]     # [2,16,256,2048]
    xs_flat = xs.reshape(PAIRS, L, D)

    # q/k lhsT tiles packed partition-major: wt[c, p, dt*128+j] =
    # WqkvT[dt*128+p, c*128+j] so one chunk is a single linear DMA.
    wt = np.ascontiguousarray(
        Wqkv[:2 * D].reshape(NQK, 128, DT, 128).transpose(0, 3, 2, 1)
        .reshape(NQK, 128, DT * 128)
    ).astype(bf16)                                          # [32,128,2048]
    # v moving quarters, same layout as the Wout quarters.
    wvt = _quarter_tiles(Wqkv[2 * D:])                      # [4,128,DT*512]
    bqt = np.ascontiguousarray(b_qkv[:2 * D].reshape(NQK, 128).T)  # [128,32]

    in_maps = []
    for i in range(N_CORES):
        tok = xs_flat[SPC * i:SPC * (i + 1)].reshape(TOK, D)
        xst = np.ascontiguousarray(
            tok.T.reshape(DT, 128, TOK).transpose(1, 0, 2)
            .reshape(128, DT * TOK)).astype(bf16)
        in_maps.append({"xst": xst, "wqkv_t": wt, "wv_t": wvt, "bq_t": bqt})
    return in_maps


def _quarter_tiles(W):
    """[2048, 2048] row-major -> [eq, p, dt*512+j] = W[eq*512+j, dt*128+p]."""
    W = np.asarray(W, dtype=np.float32)
    return np.ascontiguousarray(
        W.T.reshape(DT, 128, 4, 512).transpose(2, 1, 0, 3)
        .reshape(4, 128, DT * 512)).astype(ml_dtypes.bfloat16)


def make_wout_tiled(Wout):
    return _quarter_tiles(Wout)


def kernel(x, Wqkv, b_qkv, Wout, b_out):
    from concourse import bass_utils

    nc = get_program()
    in_maps = make_in_maps(x, Wqkv, b_qkv)
    wot = make_wout_tiled(Wout)
    for m in in_maps:
        m["wout_t"] = wot

    res = bass_utils.run_bass_kernel_spmd(
        nc, in_maps, core_ids=list(range(N_CORES)))
    outs = [res.results[i]["out"] for i in range(N_CORES)]
    # Softmax weights sum to 1, so the v bias passes through attention
    # unchanged: out += Wout @ b_v + b_out (host-side, exact).
    bias = (np.asarray(Wout, dtype=np.float32)
            @ np.asarray(b_qkv, dtype=np.float32)[2 * D:]
            + np.asarray(b_out, dtype=np.float32))
    full = np.concatenate(outs, axis=0) + bias
    return np.ascontiguousarray(full.reshape(B, NSEG * L, D), dtype=np.float32)


# revision 31
# speedup vs baseline: 1.0327x; 1.0022x over previous
"""Dilated segment attention on 8 TRN2 NeuronCores (Bass/Tile).

Problem (hardcoded from spec):
  x [2, 8192, 2048] f32, Wqkv [6144, 2048], b_qkv [6144], Wout [2048, 2048],
  b_out [2048].  segment=512, dilation=2 -> 16 segments of L=256 dilated
  tokens per batch; per-segment 16-head attention (hd=128); fused qkv and
  out projections.  Output [2, 4096, 2048] f32.

Sharding: the 32 (batch, segment) instances are independent -> 4 per core.
Host pre-gathers the dilated tokens, pre-transposes/pre-tiles operands and
casts to bf16 (compute precision; measured end-to-end rel err ~5e-3).

Per-core dataflow (all matmuls K=128, bf16):
  q/k proj   : feature-major  qkT[e, tok] = W-tile.T @ xsT-tile (accum 16 d-tiles)
  v proj     : token-major    v[tok, e]   = xsT-tile.T @ WvT-quarter
               (drains land directly in the AV-stationary layout — no
               transposes; DMA transposes act as global DMA barriers in
               Tile's deadlock-avoidance and measurably stall the PE)
  scores     : scoresT[lk, lq] = kT.T @ qT  (per seg, head; operands swapped)
  softmax    : exp on ScalarE (scale=1/sqrt(hd); scores provably in [-6, 6]
               so no max subtraction), sums via ones-matmul, normalize on DVE
  AV         : outT[hd, lq] = v[lk, hd].T @ expT[lk, lq]
  out proj   : out[l, e] = aT-tile.T @ WoutT-tile  (accum 16 head-tiles,
               token-major, so the HBM store is linear)
b_qkv's q/k parts are applied on-chip (they feed the softmax); its v part
and b_out are applied on the host: softmax weights sum to 1, so a v bias
shifts the attention output by exactly b_v, i.e. out += Wout @ b_v + b_out.

Schedule (v3 — trace-driven):
  - ~12 zero matmuls at kernel start (overlapping the input DMA wait) keep
    the PE HAM busy-window warm so real matmuls run at 2.4GHz immediately.
  - All startup loads on one HWDGE ring, interleaved [wv0/4, xq0, wv0/4,
    xq1, ...] so the first v-projection group ramps with the arrivals.
  - psum->sbuf drains are split/alternated ScalarE / DVE so chunk-boundary
    psum WAR frees faster and neither engine's FIFO gates the PE.
  - attention for head h-1 is interleaved between head h's projection
    matmuls (1 step per 3 d-tiles): the exp ACT latency and the per-tile
    LDWEIGHTS hide under 216ns projection matmuls.  Head 15's attention
    interleaves into the first out-projection psum groups.
"""

import numpy as np
import ml_dtypes

B = 2
S = 8192
D = 2048
H = 16
HD = 128
SEGMENT = 512
DIL = 2
NSEG = S // SEGMENT          # 16
L = SEGMENT // DIL           # 256 dilated tokens per segment
N_CORES = 8
PAIRS = B * NSEG             # 32 independent (b, n) instances
SPC = PAIRS // N_CORES       # 4 segments per core
TOK = SPC * L                # 1024 tokens per core
DT = D // 128                # 16 contraction tiles
NQK = 2 * D // 128           # 32 q/k feature chunks (16 q, 16 k)
SCALE = 1.0 / float(np.sqrt(HD))

_PROGRAM = None


def _build_program():
    import concourse.bass as bass
    import concourse.bacc as bacc
    import concourse.tile as tile
    from concourse import mybir

    BF = mybir.dt.bfloat16
    F32 = mybir.dt.float32
    ts = bass.ts
    IDENT = mybir.ActivationFunctionType.Identity
    EXP = mybir.ActivationFunctionType.Exp

    nc = bacc.Bacc("TRN2", target_bir_lowering=False, debug=False,
                   num_devices=N_CORES)

    xst_d = nc.dram_tensor("xst", [128, DT, TOK], BF, kind="ExternalInput")
    wqkv_d = nc.dram_tensor("wqkv_t", [NQK, 128, DT * 128], BF,
                            kind="ExternalInput")
    wv_d = nc.dram_tensor("wv_t", [4, 128, DT * 512], BF, kind="ExternalInput")
    wout_d = nc.dram_tensor("wout_t", [4, 128, DT * 512], BF, kind="ExternalInput")
    bq_d = nc.dram_tensor("bq_t", [128, NQK], F32, kind="ExternalInput")
    out_d = nc.dram_tensor("out", [TOK, D], F32, kind="ExternalOutput")

    with tile.TileContext(nc) as tc:
        with (
            tc.tile_pool(name="const", bufs=1) as const_p,
            tc.tile_pool(name="big", bufs=1) as big_p,
            tc.tile_pool(name="wq", bufs=6) as w_p,
            tc.tile_pool(name="qk", bufs=4) as qk_p,
            tc.tile_pool(name="ex", bufs=4) as ex_p,
            tc.tile_pool(name="st", bufs=2) as st_p,
            tc.tile_pool(name="ou", bufs=3) as ou_p,
            tc.tile_pool(name="pp", bufs=4, space="PSUM") as pp_p,
            tc.tile_pool(name="pa", bufs=2, space="PSUM") as pa_p,
        ):
            # All-ones stationary for the softmax sums: a [128, 128] ones
            # matrix replicates the column sums across all 128 output
            # partitions, so the reciprocal needs no partition_broadcast
            # and the matmul output drains full-width (1-row outputs incur
            # a serial-drain penalty, measured +100ns on the matmul AND on
            # the instruction after it).
            ones = const_p.tile([128, 128], BF)
            nc.gpsimd.memset(ones[:], 1.0)
            # PE warmup: the HAM clock gate holds the PE at 1.2GHz until
            # ~3.4us of sustained activity.  Burn that window on zero
            # matmuls while the input DMAs stream, so the first real
            # matmul runs at 2.4GHz.
            warm_sb = const_p.tile([128, 512], BF)
            nc.gpsimd.memset(warm_sb[:], 0.0)
            # Sized so the PE stays busy from engine-start (~8.5us) until
            # the first input DMA completes (~15.4us, fixed dispatch +
            # transfer + completion-receipt latency): ~8 matmuls run cold
            # (427ns) before HAM unthrottles, the rest warm (216ns).
            warm_ps = pp_p.tile([128, 512], F32, tag="pp", name="warm")
            for i in range(18):
                nc.tensor.matmul(warm_ps[:], warm_sb[:, 0:128], warm_sb[:],
                                 start=(i == 0), stop=(i == 17))

            # Startup loads: one FIFO ring ordered to match the first
            # v-wave's consumption exactly — [wv0/4, xq-quarter first
            # halves]x4, then the second halves.  The first 16 matmuls need
            # only 1MB; each further d-tile quarter's (weights, tokens 0-511)
            # pair lands one compute-burst ahead of its use.
            wv_tiles = [None] * 4
            wv_tiles[0] = w_p.tile([128, DT, 512], BF, tag="wo", bufs=2,
                                   name="wv_t")
            # Token-halves are separate tiles so each consumer's dependency
            # binds to exactly one DMA (subtile tracking proved coarse).
            xh_sb = [[big_p.tile([128, 4, 512], BF, name=f"xq{k}h{hf}")
                      for hf in range(2)] for k in range(4)]
            bq_sb = const_p.tile([128, NQK], F32)
            nc.scalar.dma_start(out=bq_sb[:], in_=bq_d[:])
            for k in range(4):
                nc.sync.dma_start(out=wv_tiles[0][:, 4 * k:4 * (k + 1), :],
                                  in_=wv_d[0][:, 4 * k * 512:4 * (k + 1) * 512])
                nc.sync.dma_start(out=xh_sb[k][0][:],
                                  in_=xst_d[:, 4 * k:4 * (k + 1), 0:512])
            for k in range(4):
                nc.sync.dma_start(out=xh_sb[k][1][:],
                                  in_=xst_d[:, 4 * k:4 * (k + 1), 512:TOK])
            vtok_sb = big_p.tile([128, H, SPC * 2, 128], BF)
            aT_sb = big_p.tile([128, SPC, H, L], BF)

            # ---- v projection, token-major (out-proj-style groups) ----
            # vtok[p, h, t, j] = v[tok = t*128+p, hd = j]: each (vc, t)
            # group accumulates [128 tok, 512 v-feats] over the 16 d-tiles
            # and drains straight into the AV-stationary layout.
            def v_drain(vc, t, vps):
                dst = vtok_sb[:, 4 * vc:4 * (vc + 1), t, :]
                if t % 2:
                    nc.vector.tensor_copy(out=dst, in_=vps[:])
                else:
                    nc.scalar.activation(out=dst, in_=vps[:], func=IDENT,
                                         scale=1.0)

            # Double-buffered wv prefetch: vc+1's 2MB load is dispatched at
            # the start of vc's compute.
            wv_tiles[1] = w_p.tile([128, DT, 512], BF, tag="wo", bufs=2,
                                   name="wv_t")
            nc.sync.dma_start(out=wv_tiles[1][:], in_=wv_d[1])

            # vc0 rides the input-DMA ramp: sweep d-tile-quarter-major
            # across 4 concurrent token-tile psum groups, so every quarter
            # arrival unlocks 16 matmuls instead of 4 (the in-order PE
            # stream would otherwise head-of-line block on group 0's next
            # quarter while groups 1-7's early d-tiles are ready).
            for wave in range(2):
                vpss = [pp_p.tile([128, 512], F32, tag="pp", name="vps")
                        for _ in range(4)]
                for qk in range(4):
                    for ti in range(4):
                        for r in range(4):
                            nc.tensor.matmul(
                                vpss[ti][:],
                                xh_sb[qk][wave][:, r, ti * 128:
                                                (ti + 1) * 128],
                                wv_tiles[0][:, 4 * qk + r, :],
                                start=(qk == 0 and r == 0),
                                stop=(qk == 3 and r == 3),
                            )
                for ti in range(4):
                    v_drain(0, 4 * wave + ti, vpss[ti])

            for vc in range(1, 4):
                if vc + 1 < 4:
                    wv_tiles[vc + 1] = w_p.tile([128, DT, 512], BF, tag="wo",
                                                bufs=2, name="wv_t")
                    nc.sync.dma_start(out=wv_tiles[vc + 1][:],
                                      in_=wv_d[vc + 1])
                for t in range(SPC * 2):
                    vps = pp_p.tile([128, 512], F32, tag="pp", name="vps")
                    for dt in range(DT):
                        q, r = divmod(dt, 4)
                        nc.tensor.matmul(
                            vps[:],
                            xh_sb[q][t // 4][:, r, (t % 4) * 128:
                                             (t % 4 + 1) * 128],
                            wv_tiles[vc][:, dt, :],
                            start=(dt == 0),
                            stop=(dt == DT - 1),
                        )
                    v_drain(vc, t, vps)

            def proj_steps(c, out_tile):
                """q/k chunk c: out_tile[128, TOK] bf16 = (W chunk).T @ xsT + b.

                Generator: yields after each d-tile's matmul pair; emits the
                split-engine psum drains on exhaustion.
                """
                wck = w_p.tile([128, DT * 128], BF, tag="w")
                nc.sync.dma_start(out=wck[:], in_=wqkv_d[c])
                pss = [pp_p.tile([128, 512], F32, tag="pp", name=f"ps{half}")
                       for half in range(2)]
                for dt in range(DT):
                    q, r = divmod(dt, 4)
                    for half in range(2):
                        nc.tensor.matmul(
                            pss[half][:],
                            wck[:, ts(dt, 128)],
                            xh_sb[q][half][:, r, :],
                            start=(dt == 0),
                            stop=(dt == DT - 1),
                        )
                    yield
                nc.scalar.activation(
                    out=out_tile[:, ts(0, 512)], in_=pss[0][:],
                    func=IDENT, bias=bq_sb[:, c:c + 1], scale=1.0,
                )
                nc.vector.tensor_scalar_add(
                    out_tile[:, ts(1, 512)], pss[1][:], bq_sb[:, c:c + 1],
                )

            # ---- per-head attention, interleaved into the next head's
            # projection matmul stream ----
            def attention_thunks(h, qh, kh):
                """16 emission steps for head h's attention over 4 segments.

                Each step issues at most ONE new PE stationary (scores pair
                shares the q moving operand; sums pair shares the ones
                stationary after LDWEIGHTS dedupe), so with >=2 projection
                matmuls between steps every attention LDWEIGHTS loads into
                the background weight buffer under a 216ns projection
                matmul instead of stalling the PE (measured +90..160ns on
                every av-group matmul when sums+AV+AV issued back-to-back).
                """
                e_ts = [None] * SPC
                scts = [None] * SPC
                avss = [None] * SPC

                def sc_step(seg):
                    def emit():
                        scT = scts[seg] = pa_p.tile([128, 2, L], F32,
                                                    tag="pa", name="scT")
                        for lkc in range(2):
                            nc.tensor.matmul(
                                scT[:, lkc, :],
                                kh[:, seg * L + lkc * 128:
                                   seg * L + (lkc + 1) * 128],
                                qh[:, seg * L:(seg + 1) * L],
                            )
                        e_t = ex_p.tile([128, 2, L], BF, tag="ex")
                        nc.scalar.activation(out=e_t[:], in_=scT[:],
                                             func=EXP, scale=SCALE)
                        e_ts[seg] = e_t
                    return emit

                def sums_step(seg):
                    def emit():
                        # av ([:, 0, :]) and the replicated softmax sums
                        # ([:, 1, :]) share one PSUM bank; Tile serializes
                        # the DVE reads behind all four matmul writes, which
                        # the mul needs anyway.
                        avs = avss[seg] = pa_p.tile([128, 2, L], F32,
                                                    tag="pav", bufs=2,
                                                    name="avs")
                        for lkc in range(2):
                            nc.tensor.matmul(
                                avs[:, 1, :], ones[:], e_ts[seg][:, lkc, :],
                                start=(lkc == 0), stop=(lkc == 1),
                            )
                    return emit

                def av_step(seg, lkc):
                    def emit():
                        avs = avss[seg]
                        nc.tensor.matmul(
                            avs[:, 0, :],
                            vtok_sb[:, h, seg * 2 + lkc, :],
                            e_ts[seg][:, lkc, :],
                            start=(lkc == 0), stop=(lkc == 1),
                        )
                        if lkc == 1:
                            invB = ex_p.tile([128, L], F32, tag="invb",
                                             name="invB")
                            nc.vector.reciprocal_approx_fast(
                                out=invB[:], in_=avs[:, 1, :])
                            nc.vector.tensor_mul(aT_sb[:, seg, h, :],
                                                 avs[:, 0, :], invB[:])
                    return emit

                return [sc_step(0), sc_step(1), sc_step(2),
                        sums_step(0), av_step(0, 0), av_step(0, 1),
                        sums_step(1), av_step(1, 0), av_step(1, 1),
                        sc_step(3),
                        sums_step(2), av_step(2, 0), av_step(2, 1),
                        sums_step(3), av_step(3, 0), av_step(3, 1)]

            def run_interleaved(gens, thunks, every):
                k, ai = 0, 0
                for g in gens:
                    for _ in g:
                        k += 1
                        if k % every == 0 and ai < len(thunks):
                            thunks[ai]()
                            ai += 1
                while ai < len(thunks):
                    thunks[ai]()
                    ai += 1

            wq_eq0 = None
            prev_qk = None
            for h in range(H):
                if h == H - 1:
                    # Prefetch the first Wout quarter one head early so the
                    # out-projection never waits on its 2MB load.
                    wq_eq0 = w_p.tile([128, DT, 512], BF, tag="wo", bufs=2,
                                      name="wq_t")
                    nc.sync.dma_start(out=wq_eq0[:], in_=wout_d[0])
                qh = qk_p.tile([128, TOK], BF, tag="qk")
                kh = qk_p.tile([128, TOK], BF, tag="qk")
                gens = [proj_steps(h, qh), proj_steps(16 + h, kh)]
                thunks = (attention_thunks(h - 1, *prev_qk)
                          if prev_qk is not None else [])
                run_interleaved(gens, thunks, every=2)
                prev_qk = (qh, kh)
            last_attn = attention_thunks(H - 1, *prev_qk)

            # ---- output projection (token-major) ----
            # Wout streamed in four 2MB e-quarters; head 15's attention
            # steps interleave into the first psum group (its aT d-tile is
            # the last accumulated, so each segment's normalize only has to
            # beat d-tile 15 of its own token tile).
            def po_steps(eq, lc, wq_t):
                seg, lqc = lc // 2, lc % 2
                po = pp_p.tile([128, 512], F32, tag="pp", name="po")
                for dt in range(DT):
                    nc.tensor.matmul(
                        po[:],
                        aT_sb[:, seg, dt, ts(lqc, 128)],
                        wq_t[:, dt, :],
                        start=(dt == 0),
                        stop=(dt == DT - 1),
                    )
                    yield
                ob = ou_p.tile([128, 512], F32, tag="ou")
                if lc % 2:
                    nc.vector.tensor_copy(out=ob[:], in_=po[:])
                else:
                    nc.scalar.activation(out=ob[:], in_=po[:], func=IDENT,
                                         scale=1.0)
                nc.sync.dma_start(
                    out=out_d[lc * 128:(lc + 1) * 128,
                              eq * 512:(eq + 1) * 512],
                    in_=ob[:],
                )

            for eq in range(4):
                if eq == 0:
                    wq_t = wq_eq0
                else:
                    wq_t = w_p.tile([128, DT, 512], BF, tag="wo", bufs=2,
                                    name="wq_t")
                    nc.sync.dma_start(out=wq_t[:], in_=wout_d[eq])
                for lc in range(TOK // 128):
                    if eq == 0 and lc < 2:
                        thunks = last_attn[8 * lc:8 * (lc + 1)]
                    else:
                        thunks = []
                    run_interleaved([po_steps(eq, lc, wq_t)], thunks,
                                    every=2)

    nc.compile()
    _dedupe_ldweights(nc)
    return nc


def _dedupe_ldweights(nc):
    """Drop InstLdweights whose weights are already resident in the PE array.

    tile_legalize emits one LDWEIGHTS per matmul; consecutive matmuls that
    share the stationary operand (projection token-halves) reload identical
    weights, costing ~97ns of PE pipe each.  Walk each block's PE stream
    tracking the loaded-weights key and delete reloads.  Only semaphore-free
    LDWEIGHTS are dropped, so the sync graph is untouched;
    EVENT_SEMAPHORE/DRAIN between pairs don't disturb the array, any other
    PE instruction conservatively invalidates the key.
    """
    from concourse import mybir

    PE = mybir.EngineType.PE
    dropped = 0
    for f in nc.m.functions:
        for blk in f.blocks:
            insts = blk.instructions
            loaded = None
            to_drop = []
            for idx, x in enumerate(insts):
                if getattr(x, "engine", None) != PE:
                    continue
                nm = type(x).__name__
                if nm == "InstLdweights":
                    si = x.sync_info
                    clean = si is None or (not si.on_wait and not si.on_update)
                    key = (str(x.ins[0]), str(x.is_transpose),
                           str(x.perf_mode), str(x.tile_position))
                    if clean and loaded == key:
                        to_drop.append(idx)
                    else:
                        loaded = key
                elif nm == "InstMatmult":
                    continue
                elif nm in ("InstEventSemaphore", "InstDrain"):
                    continue
                else:
                    loaded = None
            for idx in reversed(to_drop):
                del insts[idx]
            blk.instructions = insts
            dropped += len(to_drop)
    return dropped


def get_program():
    global _PROGRAM
    if _PROGRAM is None:
        _PROGRAM = _build_program()
    return _PROGRAM


def make_in_maps(x, Wqkv, b_qkv):
    """Host-side shard + layout prep (bf16 casts, transposes, tiling)."""
    bf16 = ml_dtypes.bfloat16
    x = np.asarray(x, dtype=np.float32)
    Wqkv = np.asarray(Wqkv, dtype=np.float32)
    b_qkv = np.asarray(b_qkv, dtype=np.float32)

    xs = x.reshape(B, NSEG, SEGMENT, D)[:, :, ::DIL, :]     # [2,16,256,2048]
    xs_flat = xs.reshape(PAIRS, L, D)

    # q/k lhsT tiles packed partition-major: wt[c, p, dt*128+j] =
    # WqkvT[dt*128+p, c*128+j] so one chunk is a single linear DMA.
    wt = np.ascontiguousarray(
        Wqkv[:2 * D].reshape(NQK, 128, DT, 128).transpose(0, 3, 2, 1)
        .reshape(NQK, 128, DT * 128)
    ).astype(bf16)                                          # [32,128,2048]
    # v moving quarters, same layout as the Wout quarters.
    wvt = _quarter_tiles(Wqkv[2 * D:])                      # [4,128,DT*512]
    bqt = np.ascontiguousarray(b_qkv[:2 * D].reshape(NQK, 128).T)  # [128,32]

    in_maps = []
    for i in range(N_CORES):
        tok = xs_flat[SPC * i:SPC * (i + 1)].reshape(TOK, D)
        xst = np.ascontiguousarray(
            tok.T.reshape(DT, 128, TOK).transpose(1, 0, 2)
            .reshape(128, DT * TOK)).astype(bf16)
        in_maps.append({"xst": xst, "wqkv_t": wt, "wv_t": wvt, "bq_t": bqt})
    return in_maps


def _quarter_tiles(W):
    """[2048, 2048] row-major -> [eq, p, dt*512+j] = W[eq*512+j, dt*128+p]."""
    W = np.asarray(W, dtype=np.float32)
    return np.ascontiguousarray(
        W.T.reshape(DT, 128, 4, 512).transpose(2, 1, 0, 3)
        .reshape(4, 128, DT * 512)).astype(ml_dtypes.bfloat16)


def make_wout_tiled(Wout):
    return _quarter_tiles(Wout)


def kernel(x, Wqkv, b_qkv, Wout, b_out):
    from concourse import bass_utils

    nc = get_program()
    in_maps = make_in_maps(x, Wqkv, b_qkv)
    wot = make_wout_tiled(Wout)
    for m in in_maps:
        m["wout_t"] = wot

    res = bass_utils.run_bass_kernel_spmd(
        nc, in_maps, core_ids=list(range(N_CORES)))
    outs = [res.results[i]["out"] for i in range(N_CORES)]
    # Softmax weights sum to 1, so the v bias passes through attention
    # unchanged: out += Wout @ b_v + b_out (host-side, exact).
    bias = (np.asarray(Wout, dtype=np.float32)
            @ np.asarray(b_qkv, dtype=np.float32)[2 * D:]
            + np.asarray(b_out, dtype=np.float32))
    full = np.concatenate(outs, axis=0) + bias
    return np.ascontiguousarray(full.reshape(B, NSEG * L, D), dtype=np.float32)


# revision 32
# speedup vs baseline: 1.0336x; 1.0009x over previous
"""Dilated segment attention on 8 TRN2 NeuronCores (Bass/Tile).

Problem (hardcoded from spec):
  x [2, 8192, 2048] f32, Wqkv [6144, 2048], b_qkv [6144], Wout [2048, 2048],
  b_out [2048].  segment=512, dilation=2 -> 16 segments of L=256 dilated
  tokens per batch; per-segment 16-head attention (hd=128); fused qkv and
  out projections.  Output [2, 4096, 2048] f32.

Sharding: the 32 (batch, segment) instances are independent -> 4 per core.
Host pre-gathers the dilated tokens, pre-transposes/pre-tiles operands and
casts to bf16 (compute precision; measured end-to-end rel err ~5e-3).

Per-core dataflow (all matmuls K=128, bf16):
  q/k proj   : feature-major  qkT[e, tok] = W-tile.T @ xsT-tile (accum 16 d-tiles)
  v proj     : token-major    v[tok, e]   = xsT-tile.T @ WvT-quarter
               (drains land directly in the AV-stationary layout — no
               transposes; DMA transposes act as global DMA barriers in
               Tile's deadlock-avoidance and measurably stall the PE)
  scores     : scoresT[lk, lq] = kT.T @ qT  (per seg, head; operands swapped)
  softmax    : exp on ScalarE (scale=1/sqrt(hd); scores provably in [-6, 6]
               so no max subtraction); sums via an all-ones [128,128]
               stationary so they land replicated across partitions (no
               partition_broadcast, full-width psum drain); reciprocal +
               normalize on DVE
  AV         : outT[hd, lq] = v[lk, hd].T @ expT[lk, lq]
  out proj   : out[l, e] = aT-tile.T @ WoutT-tile  (accum 16 head-tiles,
               token-major, so the HBM store is linear)
b_qkv's q/k parts are applied on-chip (they feed the softmax); its v part
and b_out are applied on the host: softmax weights sum to 1, so a v bias
shifts the attention output by exactly b_v, i.e. out += Wout @ b_v + b_out.

Schedule (trace-driven, measured on HW):
  - ~18 zero matmuls at kernel start (overlapping the ~15us fixed input
    DMA latency) keep the PE HAM busy-window warm so real matmuls run at
    2.4GHz immediately.
  - All startup loads on one HWDGE ring, ordered to match the first
    v-wave's consumption ([wv0/4 + xq-quarter token-half] pairs); the
    first v quarter sweeps d-tile-quarter-major across 4 concurrent
    token-tile psum groups so the PE rides the DMA ramp.
  - psum->sbuf drains are split/alternated ScalarE / DVE so chunk-boundary
    psum WAR frees faster and neither engine's FIFO gates the PE.
  - attention for head h-1 is interleaved between head h's projection
    matmuls, one single-stationary step per 2 d-tiles: the exp ACT latency
    and every attention LDWEIGHTS hide under 216ns projection matmuls.
    Head 15's attention interleaves into the first out-projection groups.
"""

import numpy as np
import ml_dtypes

B = 2
S = 8192
D = 2048
H = 16
HD = 128
SEGMENT = 512
DIL = 2
NSEG = S // SEGMENT          # 16
L = SEGMENT // DIL           # 256 dilated tokens per segment
N_CORES = 8
PAIRS = B * NSEG             # 32 independent (b, n) instances
SPC = PAIRS // N_CORES       # 4 segments per core
TOK = SPC * L                # 1024 tokens per core
DT = D // 128                # 16 contraction tiles
NQK = 2 * D // 128           # 32 q/k feature chunks (16 q, 16 k)
SCALE = 1.0 / float(np.sqrt(HD))

_PROGRAM = None


def _build_program():
    import concourse.bass as bass
    import concourse.bacc as bacc
    import concourse.tile as tile
    from concourse import mybir

    BF = mybir.dt.bfloat16
    F32 = mybir.dt.float32
    ts = bass.ts
    IDENT = mybir.ActivationFunctionType.Identity
    EXP = mybir.ActivationFunctionType.Exp

    nc = bacc.Bacc("TRN2", target_bir_lowering=False, debug=False,
                   num_devices=N_CORES)

    xst_d = nc.dram_tensor("xst", [128, DT, TOK], BF, kind="ExternalInput")
    wqkv_d = nc.dram_tensor("wqkv_t", [NQK, 128, DT * 128], BF,
                            kind="ExternalInput")
    wv_d = nc.dram_tensor("wv_t", [4, 128, DT * 512], BF, kind="ExternalInput")
    wout_d = nc.dram_tensor("wout_t", [4, 128, DT * 512], BF, kind="ExternalInput")
    bq_d = nc.dram_tensor("bq_t", [128, NQK], F32, kind="ExternalInput")
    out_d = nc.dram_tensor("out", [TOK, D], F32, kind="ExternalOutput")

    with tile.TileContext(nc) as tc:
        with (
            tc.tile_pool(name="const", bufs=1) as const_p,
            tc.tile_pool(name="big", bufs=1) as big_p,
            tc.tile_pool(name="wq", bufs=6) as w_p,
            tc.tile_pool(name="qk", bufs=4) as qk_p,
            tc.tile_pool(name="ex", bufs=4) as ex_p,
            tc.tile_pool(name="st", bufs=2) as st_p,
            tc.tile_pool(name="ou", bufs=3) as ou_p,
            tc.tile_pool(name="pp", bufs=4, space="PSUM") as pp_p,
            tc.tile_pool(name="pa", bufs=2, space="PSUM") as pa_p,
        ):
            # All-ones stationary for the softmax sums: a [128, 128] ones
            # matrix replicates the column sums across all 128 output
            # partitions, so the reciprocal needs no partition_broadcast
            # and the matmul output drains full-width (1-row outputs incur
            # a serial-drain penalty, measured +100ns on the matmul AND on
            # the instruction after it).
            ones = const_p.tile([128, 128], BF)
            nc.gpsimd.memset(ones[:], 1.0)
            # PE warmup: the HAM clock gate holds the PE at 1.2GHz until
            # ~3.4us of sustained activity.  Burn that window on zero
            # matmuls while the input DMAs stream, so the first real
            # matmul runs at 2.4GHz.
            warm_sb = const_p.tile([128, 512], BF)
            nc.gpsimd.memset(warm_sb[:], 0.0)
            # Sized so the PE stays busy from engine-start (~8.5us) until
            # the first input DMA completes (~15.4us, fixed dispatch +
            # transfer + completion-receipt latency): ~8 matmuls run cold
            # (427ns) before HAM unthrottles, the rest warm (216ns).
            warm_ps = pp_p.tile([128, 512], F32, tag="pp", name="warm")
            for i in range(18):
                nc.tensor.matmul(warm_ps[:], warm_sb[:, 0:128], warm_sb[:],
                                 start=(i == 0), stop=(i == 17))

            # Startup loads: one FIFO ring ordered to match the first
            # v-wave's consumption exactly — [wv0/4, xq-quarter first
            # halves]x4, then the second halves.  The first 16 matmuls need
            # only 1MB; each further d-tile quarter's (weights, tokens 0-511)
            # pair lands one compute-burst ahead of its use.
            wv_tiles = [None] * 4
            wv_tiles[0] = w_p.tile([128, DT, 512], BF, tag="wo", bufs=2,
                                   name="wv_t")
            # Token-halves are separate tiles so each consumer's dependency
            # binds to exactly one DMA (subtile tracking proved coarse).
            xh_sb = [[big_p.tile([128, 4, 512], BF, name=f"xq{k}h{hf}")
                      for hf in range(2)] for k in range(4)]
            bq_sb = const_p.tile([128, NQK], F32)
            nc.scalar.dma_start(out=bq_sb[:], in_=bq_d[:])
            for k in range(4):
                nc.sync.dma_start(out=wv_tiles[0][:, 4 * k:4 * (k + 1), :],
                                  in_=wv_d[0][:, 4 * k * 512:4 * (k + 1) * 512])
                nc.sync.dma_start(out=xh_sb[k][0][:],
                                  in_=xst_d[:, 4 * k:4 * (k + 1), 0:512])
            for k in range(4):
                nc.sync.dma_start(out=xh_sb[k][1][:],
                                  in_=xst_d[:, 4 * k:4 * (k + 1), 512:TOK])
            vtok_sb = big_p.tile([128, H, SPC * 2, 128], BF)
            aT_sb = big_p.tile([128, SPC, H, L], BF)

            # ---- v projection, token-major (out-proj-style groups) ----
            # vtok[p, h, t, j] = v[tok = t*128+p, hd = j]: each (vc, t)
            # group accumulates [128 tok, 512 v-feats] over the 16 d-tiles
            # and drains straight into the AV-stationary layout.
            def v_drain(vc, t, vps):
                dst = vtok_sb[:, 4 * vc:4 * (vc + 1), t, :]
                if t % 2:
                    nc.vector.tensor_copy(out=dst, in_=vps[:])
                else:
                    nc.scalar.activation(out=dst, in_=vps[:], func=IDENT,
                                         scale=1.0)

            # Double-buffered wv prefetch: vc+1's 2MB load is dispatched at
            # the start of vc's compute.
            wv_tiles[1] = w_p.tile([128, DT, 512], BF, tag="wo", bufs=2,
                                   name="wv_t")
            nc.sync.dma_start(out=wv_tiles[1][:], in_=wv_d[1])

            # vc0 rides the input-DMA ramp: sweep d-tile-quarter-major
            # across 4 concurrent token-tile psum groups, so every quarter
            # arrival unlocks 16 matmuls instead of 4 (the in-order PE
            # stream would otherwise head-of-line block on group 0's next
            # quarter while groups 1-7's early d-tiles are ready).
            for wave in range(2):
                vpss = [pp_p.tile([128, 512], F32, tag="pp", name="vps")
                        for _ in range(4)]
                for qk in range(4):
                    for ti in range(4):
                        for r in range(4):
                            nc.tensor.matmul(
                                vpss[ti][:],
                                xh_sb[qk][wave][:, r, ti * 128:
                                                (ti + 1) * 128],
                                wv_tiles[0][:, 4 * qk + r, :],
                                start=(qk == 0 and r == 0),
                                stop=(qk == 3 and r == 3),
                            )
                for ti in range(4):
                    v_drain(0, 4 * wave + ti, vpss[ti])

            for vc in range(1, 4):
                if vc + 1 < 4:
                    wv_tiles[vc + 1] = w_p.tile([128, DT, 512], BF, tag="wo",
                                                bufs=2, name="wv_t")
                    nc.sync.dma_start(out=wv_tiles[vc + 1][:],
                                      in_=wv_d[vc + 1])
                for t in range(SPC * 2):
                    vps = pp_p.tile([128, 512], F32, tag="pp", name="vps")
                    for dt in range(DT):
                        q, r = divmod(dt, 4)
                        nc.tensor.matmul(
                            vps[:],
                            xh_sb[q][t // 4][:, r, (t % 4) * 128:
                                             (t % 4 + 1) * 128],
                            wv_tiles[vc][:, dt, :],
                            start=(dt == 0),
                            stop=(dt == DT - 1),
                        )
                    v_drain(vc, t, vps)

            def proj_steps(c, out_tile):
                """q/k chunk c: out_tile[128, TOK] bf16 = (W chunk).T @ xsT + b.

                Generator: yields after each d-tile's matmul pair; emits the
                split-engine psum drains on exhaustion.
                """
                wck = w_p.tile([128, DT * 128], BF, tag="w")
                nc.sync.dma_start(out=wck[:], in_=wqkv_d[c])
                pss = [pp_p.tile([128, 512], F32, tag="pp", name=f"ps{half}")
                       for half in range(2)]
                for dt in range(DT):
                    q, r = divmod(dt, 4)
                    for half in range(2):
                        nc.tensor.matmul(
                            pss[half][:],
                            wck[:, ts(dt, 128)],
                            xh_sb[q][half][:, r, :],
                            start=(dt == 0),
                            stop=(dt == DT - 1),
                        )
                    yield
                nc.scalar.activation(
                    out=out_tile[:, ts(0, 512)], in_=pss[0][:],
                    func=IDENT, bias=bq_sb[:, c:c + 1], scale=1.0,
                )
                nc.vector.tensor_scalar_add(
                    out_tile[:, ts(1, 512)], pss[1][:], bq_sb[:, c:c + 1],
                )

            # ---- per-head attention, interleaved into the next head's
            # projection matmul stream ----
            def attention_thunks(h, qh, kh):
                """16 emission steps for head h's attention over 4 segments.

                Each step issues at most ONE new PE stationary (scores pair
                shares the q moving operand; sums pair shares the ones
                stationary after LDWEIGHTS dedupe), so with >=2 projection
                matmuls between steps every attention LDWEIGHTS loads into
                the background weight buffer under a 216ns projection
                matmul instead of stalling the PE (measured +90..160ns on
                every av-group matmul when sums+AV+AV issued back-to-back).
                """
                e_ts = [None] * SPC
                scts = [None] * SPC
                avss = [None] * SPC

                def sc_step(seg):
                    def emit():
                        scT = scts[seg] = pa_p.tile([128, 2, L], F32,
                                                    tag="pa", name="scT")
                        for lkc in range(2):
                            nc.tensor.matmul(
                                scT[:, lkc, :],
                                kh[:, seg * L + lkc * 128:
                                   seg * L + (lkc + 1) * 128],
                                qh[:, seg * L:(seg + 1) * L],
                            )
                        e_t = ex_p.tile([128, 2, L], BF, tag="ex")
                        nc.scalar.activation(out=e_t[:], in_=scT[:],
                                             func=EXP, scale=SCALE)
                        e_ts[seg] = e_t
                    return emit

                def sums_step(seg):
                    def emit():
                        # av ([:, 0, :]) and the replicated softmax sums
                        # ([:, 1, :]) share one PSUM bank; Tile serializes
                        # the DVE reads behind all four matmul writes, which
                        # the mul needs anyway.
                        avs = avss[seg] = pa_p.tile([128, 2, L], F32,
                                                    tag="pav", bufs=2,
                                                    name="avs")
                        for lkc in range(2):
                            nc.tensor.matmul(
                                avs[:, 1, :], ones[:], e_ts[seg][:, lkc, :],
                                start=(lkc == 0), stop=(lkc == 1),
                            )
                    return emit

                def av_step(seg, lkc):
                    def emit():
                        avs = avss[seg]
                        nc.tensor.matmul(
                            avs[:, 0, :],
                            vtok_sb[:, h, seg * 2 + lkc, :],
                            e_ts[seg][:, lkc, :],
                            start=(lkc == 0), stop=(lkc == 1),
                        )
                        if lkc == 1:
                            invB = ex_p.tile([128, L], F32, tag="invb",
                                             name="invB")
                            nc.vector.reciprocal_approx_fast(
                                out=invB[:], in_=avs[:, 1, :])
                            nc.vector.tensor_mul(aT_sb[:, seg, h, :],
                                                 avs[:, 0, :], invB[:])
                    return emit

                return [sc_step(0), sc_step(1), sc_step(2),
                        sums_step(0), av_step(0, 0), av_step(0, 1),
                        sums_step(1), av_step(1, 0), av_step(1, 1),
                        sc_step(3),
                        sums_step(2), av_step(2, 0), av_step(2, 1),
                        sums_step(3), av_step(3, 0), av_step(3, 1)]

            def run_interleaved(gens, thunks, every):
                k, ai = 0, 0
                for g in gens:
                    for _ in g:
                        k += 1
                        if k % every == 0 and ai < len(thunks):
                            thunks[ai]()
                            ai += 1
                while ai < len(thunks):
                    thunks[ai]()
                    ai += 1

            wq_eq0 = None
            prev_qk = None
            for h in range(H):
                if h == H - 1:
                    # Prefetch the first Wout quarter one head early so the
                    # out-projection never waits on its 2MB load.
                    wq_eq0 = w_p.tile([128, DT, 512], BF, tag="wo", bufs=2,
                                      name="wq_t")
                    nc.sync.dma_start(out=wq_eq0[:], in_=wout_d[0])
                qh = qk_p.tile([128, TOK], BF, tag="qk")
                kh = qk_p.tile([128, TOK], BF, tag="qk")
                gens = [proj_steps(h, qh), proj_steps(16 + h, kh)]
                thunks = (attention_thunks(h - 1, *prev_qk)
                          if prev_qk is not None else [])
                run_interleaved(gens, thunks, every=2)
                prev_qk = (qh, kh)
            last_attn = attention_thunks(H - 1, *prev_qk)

            # ---- output projection (token-major) ----
            # Wout streamed in four 2MB e-quarters; head 15's attention
            # steps interleave into the first psum group (its aT d-tile is
            # the last accumulated, so each segment's normalize only has to
            # beat d-tile 15 of its own token tile).
            def po_steps(eq, lc, wq_t):
                seg, lqc = lc // 2, lc % 2
                po = pp_p.tile([128, 512], F32, tag="pp", name="po")
                for dt in range(DT):
                    nc.tensor.matmul(
                        po[:],
                        aT_sb[:, seg, dt, ts(lqc, 128)],
                        wq_t[:, dt, :],
                        start=(dt == 0),
                        stop=(dt == DT - 1),
                    )
                    yield
                ob = ou_p.tile([128, 512], F32, tag="ou")
                if lc % 2:
                    nc.vector.tensor_copy(out=ob[:], in_=po[:])
                else:
                    nc.scalar.activation(out=ob[:], in_=po[:], func=IDENT,
                                         scale=1.0)
                nc.sync.dma_start(
                    out=out_d[lc * 128:(lc + 1) * 128,
                              eq * 512:(eq + 1) * 512],
                    in_=ob[:],
                )

            for eq in range(4):
                if eq == 0:
                    wq_t = wq_eq0
                else:
                    wq_t = w_p.tile([128, DT, 512], BF, tag="wo", bufs=2,
                                    name="wq_t")
                    nc.sync.dma_start(out=wq_t[:], in_=wout_d[eq])
                for lc in range(TOK // 128):
                    if eq == 0 and lc < 2:
                        thunks = last_attn[8 * lc:8 * (lc + 1)]
                    else:
                        thunks = []
                    run_interleaved([po_steps(eq, lc, wq_t)], thunks,
                                    every=2)

    nc.compile()
    _dedupe_ldweights(nc)
    return nc


def _dedupe_ldweights(nc):
    """Drop InstLdweights whose weights are already resident in the PE array.

    tile_legalize emits one LDWEIGHTS per matmul; consecutive matmuls that
    share the stationary operand (projection token-halves) reload identical
    weights, costing ~97ns of PE pipe each.  Walk each block's PE stream
    tracking the loaded-weights key and delete reloads.  Only semaphore-free
    LDWEIGHTS are dropped, so the sync graph is untouched;
    EVENT_SEMAPHORE/DRAIN between pairs don't disturb the array, any other
    PE instruction conservatively invalidates the key.
    """
    from concourse import mybir

    PE = mybir.EngineType.PE
    dropped = 0
    for f in nc.m.functions:
        for blk in f.blocks:
            insts = blk.instructions
            loaded = None
            to_drop = []
            for idx, x in enumerate(insts):
                if getattr(x, "engine", None) != PE:
                    continue
                nm = type(x).__name__
                if nm == "InstLdweights":
                    si = x.sync_info
                    clean = si is None or (not si.on_wait and not si.on_update)
                    key = (str(x.ins[0]), str(x.is_transpose),
                           str(x.perf_mode), str(x.tile_position))
                    if clean and loaded == key:
                        to_drop.append(idx)
                    else:
                        loaded = key
                elif nm == "InstMatmult":
                    continue
                elif nm in ("InstEventSemaphore", "InstDrain"):
                    continue
                else:
                    loaded = None
            for idx in reversed(to_drop):
                del insts[idx]
            blk.instructions = insts
            dropped += len(to_drop)
    return dropped


def get_program():
    global _PROGRAM
    if _PROGRAM is None:
        _PROGRAM = _build_program()
    return _PROGRAM


def make_in_maps(x, Wqkv, b_qkv):
    """Host-side shard + layout prep (bf16 casts, transposes, tiling)."""
    bf16 = ml_dtypes.bfloat16
    x = np.asarray(x, dtype=np.float32)
    Wqkv = np.asarray(Wqkv, dtype=np.float32)
    b_qkv = np.asarray(b_qkv, dtype=np.float32)

    xs = x.reshape(B, NSEG, SEGMENT, D)[:, :, ::DIL, :]     # [2,16,256,2048]
    xs_flat = xs.reshape(PAIRS, L, D)

    # q/k lhsT tiles packed partition-major: wt[c, p, dt*128+j] =
    # WqkvT[dt*128+p, c*128+j] so one chunk is a single linear DMA.
    wt = np.ascontiguousarray(
        Wqkv[:2 * D].reshape(NQK, 128, DT, 128).transpose(0, 3, 2, 1)
        .reshape(NQK, 128, DT * 128)
    ).astype(bf16)                                          # [32,128,2048]
    # v moving quarters, same layout as the Wout quarters.
    wvt = _quarter_tiles(Wqkv[2 * D:])                      # [4,128,DT*512]
    bqt = np.ascontiguousarray(b_qkv[:2 * D].reshape(NQK, 128).T)  # [128,32]

    in_maps = []
    for i in range(N_CORES):
        tok = xs_flat[SPC * i:SPC * (i + 1)].reshape(TOK, D)
        xst = np.ascontiguousarray(
            tok.T.reshape(DT, 128, TOK).transpose(1, 0, 2)
            .reshape(128, DT * TOK)).astype(bf16)
        in_maps.append({"xst": xst, "wqkv_t": wt, "wv_t": wvt, "bq_t": bqt})
    return in_maps


def _quarter_tiles(W):
    """[2048, 2048] row-major -> [eq, p, dt*512+j] = W[eq*512+j, dt*128+p]."""
    W = np.asarray(W, dtype=np.float32)
    return np.ascontiguousarray(
        W.T.reshape(DT, 128, 4, 512).transpose(2, 1, 0, 3)
        .reshape(4, 128, DT * 512)).astype(ml_dtypes.bfloat16)


def make_wout_tiled(Wout):
    return _quarter_tiles(Wout)


def kernel(x, Wqkv, b_qkv, Wout, b_out):
    from concourse import bass_utils

    nc = get_program()
    in_maps = make_in_maps(x, Wqkv, b_qkv)
    wot = make_wout_tiled(Wout)
    for m in in_maps:
        m["wout_t"] = wot

    res = bass_utils.run_bass_kernel_spmd(
        nc, in_maps, core_ids=list(range(N_CORES)))
    outs = [res.results[i]["out"] for i in range(N_CORES)]
    # Softmax weights sum to 1, so the v bias passes through attention
    # unchanged: out += Wout @ b_v + b_out (host-side, exact).
    bias = (np.asarray(Wout, dtype=np.float32)
            @ np.asarray(b_qkv, dtype=np.float32)[2 * D:]
            + np.asarray(b_out, dtype=np.float32))
    full = np.concatenate(outs, axis=0) + bias
    return np.ascontiguousarray(full.reshape(B, NSEG * L, D), dtype=np.float32)
